# revision 24
# baseline (speedup 1.0000x reference)
"""Trainium2 Bass kernel for nn_Attention_78151224918608.

Dense transformer attention block: QKV proj + RoPE + GQA causal attention
+ output proj. Sharding: tensor-parallel over heads across 8 cores
(core c: Q heads 4c..4c+3, KV head c). Each core computes a partial
output (its heads through wo rows); host sums the 8 bf16 partials in
fp32 and casts to bf16.

Layout strategy (per core, per batch):
  - All matmul operands bf16; accumulation fp32 in PSUM.
  - Projections computed transposed: QKV^T[384, S] = wqkv^T @ x^T so that
    Q^T/K^T (head-dim on partitions) feed the scores matmul directly.
  - RoPE: even/odd pair interleave is folded into wq/wk/wo columns on the
    host (perm = evens-then-odds), turning the pair swap into a 32-row
    block swap done with cross-partition copies on DVE.
  - Scores computed transposed per (b,h): S^T[k,q] = K^T.T @ Q^T, so the
    softmax denominator and P@V both contract over k = partitions:
    PV lhsT = [V | ones-col] gives O^T rows 0:64 and sumexp in row 64.
  - Causal: scores/exp/PV matmuls are column-clipped to the staircase;
    diagonal 128x128 windows get a 0/1 lower-tri multiply after exp.
  - Schedule: x is DMA-streamed n-major (token-tile chunks of all 16
    d-tiles); per token tile: KV proj -> Q proj -> attention for that
    q-tile, with output-projection (wo) work for the previous q-tile
    interleaved between attention pipeline steps to keep PE fed while
    the Activation engine runs exp. b1's x load and projections overlap
    b0's attention (KT2/VT/V/OT tiles double-buffered).
"""

import sys

sys.path.insert(0, "/opt/trn_rl_repo")

import math
from collections import deque
import numpy as np
import ml_dtypes

BF16 = ml_dtypes.bfloat16

# Problem constants (hardcoded per contract).
B = 2
S = 2048
D = 2048
N_HEADS = 32
N_KV_HEADS = 8
HD = 64
N_CORES = 8
HQ = N_HEADS // N_CORES  # 4 q heads per core
M_PROJ = HQ * HD + 2 * HD  # 384: [Q0 Q1 Q2 Q3 | K | V]
QTS = 512  # q tile size (free dim)
KTS = 128  # k tile size (partitions)


def build_program(
    s=S,
    d=D,
    phase_log=None,
    lag=1,
    wo_rot=False,
    norm_pool=False,
    rope_evac_dve=False,
    mask_split=True,
):
    import concourse.bass as bass
    import concourse.mybir as mybir
    import concourse.tile as tile
    from concourse import bacc

    def mark(label):
        if phase_log is not None:
            phase_log.append((label, len(nc.inst_map)))

    f32 = mybir.dt.float32
    bf16 = mybir.dt.bfloat16
    Exp = mybir.ActivationFunctionType.Exp
    Copy = mybir.ActivationFunctionType.Copy
    add_op = mybir.AluOpType.add
    mult_op = mybir.AluOpType.mult

    n_qt = s // QTS  # q tiles per batch (4)
    n_dkt = d // 128  # contraction tiles for projections (16)
    n_skt = s // KTS  # k tiles per batch (16)
    n_mo = (HQ * HD) // 128  # wo contraction tiles (2)

    nc = bacc.Bacc("TRN2", num_devices=N_CORES)
    xT_d = nc.declare_dram_parameter("xT", [B, d, s], bf16, isOutput=False)
    wqkv_d = nc.declare_dram_parameter("wqkv", [d, M_PROJ], bf16, isOutput=False)
    wo_d = nc.declare_dram_parameter("wo_s", [HQ * HD, d], bf16, isOutput=False)
    cos_d = nc.declare_dram_parameter("cosb", [128, s], bf16, isOutput=False)
    sin_d = nc.declare_dram_parameter("sinb", [128, s], bf16, isOutput=False)
    tri_d = nc.declare_dram_parameter("tri128", [128, 128], bf16, isOutput=False)
    part_d = nc.declare_dram_parameter("part", [B * s, d], bf16, isOutput=True)

    with tile.TileContext(nc) as tc:
        with (
            tc.tile_pool(name="const", bufs=1) as cpool,
            tc.tile_pool(name="big", bufs=1) as bpool,
            tc.tile_pool(name="work", bufs=3) as wpool,
            tc.tile_pool(name="estrip", bufs=8) as epool,
            tc.tile_pool(name="outp", bufs=4) as opool,
            tc.tile_pool(name="norm", bufs=3) as rpool,
            tc.tile_pool(name="pssc", bufs=2, space="PSUM") as pssc,
            tc.tile_pool(name="psops", bufs=2, space="PSUM") as psops,
            tc.tile_pool(name="psw", bufs=2, space="PSUM") as psw,
        ):
            # ---- constants / weights ----
            cos_sb = cpool.tile([128, s], bf16)
            sin_sb = cpool.tile([128, s], bf16)
            tri_sb = cpool.tile([128, 128], bf16)
            wqkv_sb = cpool.tile([128, n_dkt, M_PROJ], bf16)
            wo_sb = cpool.tile([128, n_mo, d], bf16)

            # K/V weight columns first so the first projection can start as
            # soon as the first x chunk lands; Q columns + wo arrive behind it.
            nc.sync.dma_start(
                wqkv_sb[:, :, 256:384],
                wqkv_d[:, 256:384].rearrange("(j p) c -> p j c", p=128),
            )

            tiles = {}

            def get_batch_tiles(b):
                if ("xT", b) not in tiles:
                    tiles[("xT", b)] = bpool.tile(
                        [128, n_dkt, s], bf16, tag="xT", name=f"xT{b}"
                    )
                    tiles[("QT", b)] = bpool.tile(
                        [128, n_mo, s], bf16, tag="QT", name=f"QT{b}"
                    )
                    tiles[("KT2", b)] = bpool.tile(
                        [128, s], bf16, tag="KT2", bufs=2, name=f"KT2{b}"
                    )
                    tiles[("VT", b)] = bpool.tile(
                        [128, s], bf16, tag="VT", bufs=2, name=f"VT{b}"
                    )
                    tiles[("V", b)] = bpool.tile(
                        [128, n_skt, 128], bf16, tag="V", bufs=2, name=f"V{b}"
                    )
                    tiles[("OT", b)] = bpool.tile(
                        [128, n_mo, s], bf16, tag="OT", bufs=2, name=f"OT{b}"
                    )
                return tiles

            def load_x_chunk(b, n):
                """DMA one token-tile chunk of x^T: all d-tiles, cols nsl."""
                xT_sb = get_batch_tiles(b)[("xT", b)]
                nsl = slice(n * QTS, (n + 1) * QTS)
                nc.sync.dma_start(
                    xT_sb[:, :, nsl],
                    xT_d[b, :, nsl].rearrange("(j p) c -> p j c", p=128),
                )

            def rope_pair(dst, ps_src, rows, nsl, swaps):
                """RoPE on `rows` partitions of a psum tile into dst cols nsl."""
                r = slice(0, rows)
                q_raw = wpool.tile([128, QTS], bf16, tag="qraw")
                if rope_evac_dve:
                    nc.vector.tensor_copy(q_raw[r, :], ps_src[r, :])
                else:
                    nc.scalar.activation(q_raw[r, :], ps_src[r, :], Copy)
                t1 = wpool.tile([128, QTS], bf16, tag="t1")
                t2 = wpool.tile([128, QTS], bf16, tag="t2")
                nc.vector.tensor_tensor(t1[r, :], q_raw[r, :], cos_sb[r, nsl], mult_op)
                qsw = wpool.tile([128, QTS], bf16, tag="qsw")
                for r0, r1 in swaps:
                    nc.vector.tensor_copy(qsw[r0 : r0 + 32, :], q_raw[r1 : r1 + 32, :])
                nc.vector.tensor_tensor(t2[r, :], qsw[r, :], sin_sb[r, nsl], mult_op)
                nc.vector.tensor_tensor(dst, t1[r, :], t2[r, :], add_op)

            def kv_proj_block(b, n):
                """K/V projection for token tile n: 16 matmuls + evac."""
                mark(f"b{b}n{n}_kv")
                bt = get_batch_tiles(b)
                xT_sb = bt[("xT", b)]
                KT2_sb = bt[("KT2", b)]
                VT_sb = bt[("VT", b)]
                V_sb = bt[("V", b)]
                nsl = slice(n * QTS, (n + 1) * QTS)
                if n == 0:
                    # ones column / zero pad for PV lhsT
                    nc.gpsimd.memset(V_sb[:, :, 64:128], 0.0)
                    nc.gpsimd.memset(V_sb[:, :, 64:65], 1.0)
                ps = psw.tile([128, QTS], f32, tag="w")
                for kt in range(n_dkt):
                    nc.tensor.matmul(
                        ps[:],
                        wqkv_sb[:, kt, 256:384],
                        xT_sb[:, kt, nsl],
                        start=(kt == 0),
                        stop=(kt == n_dkt - 1),
                    )
                # rows 0:64 = K^T (rope), rows 64:128 = V^T (copy)
                rope_pair(KT2_sb[0:64, nsl], ps, 64, nsl, ((0, 32), (32, 0)))
                # duplicate K^T into partitions 64:128 (row-group packing)
                nc.vector.tensor_copy(KT2_sb[64:128, nsl], KT2_sb[0:64, nsl])
                # V^T: plain cast copy into partitions 64:128
                nc.scalar.activation(VT_sb[64:128, nsl], ps[64:128, :], Copy)
                # V^T -> V (token-major) via DMA transpose
                for kt in range(n * 4, n * 4 + 4):
                    nc.sync.dma_start_transpose(
                        V_sb[:, kt, 0:64],
                        VT_sb[64:128, kt * KTS : (kt + 1) * KTS],
                    )

            def q_proj_block(b, n, m):
                """Q projection for head pair m (heads 2m, 2m+1), token tile n."""
                bt = get_batch_tiles(b)
                xT_sb = bt[("xT", b)]
                QT_sb = bt[("QT", b)]
                nsl = slice(n * QTS, (n + 1) * QTS)
                ps = psw.tile([128, QTS], f32, tag="w")
                for kt in range(n_dkt):
                    nc.tensor.matmul(
                        ps[:],
                        wqkv_sb[:, kt, m * 128 : (m + 1) * 128],
                        xT_sb[:, kt, nsl],
                        start=(kt == 0),
                        stop=(kt == n_dkt - 1),
                    )
                rope_pair(
                    QT_sb[:, m, nsl], ps, 128, nsl, ((0, 32), (32, 0), (64, 96), (96, 64))
                )

            # ---- wo filler machinery ----
            wo_queue = deque()

            def wo_unit(b, mt, nw, drain=False):
                """One wo output tile [128 tokens, 512 d-cols]."""
                OT_sb = tiles[("OT", b)]
                msl = slice(mt * 128, (mt + 1) * 128)
                nsl = slice(nw * QTS, (nw + 1) * QTS)
                osb = tiles.get(("osb", b, mt))
                if osb is None:
                    osb = opool.tile([128, d], bf16, tag="osb", name=f"osb{b}_{mt}")
                    tiles[("osb", b, mt)] = osb
                if drain:
                    # attention PSUM pools are idle during the final drain;
                    # borrow them so more units can be in flight
                    pool, tg = ((pssc, "sc"), (psops, "ops"), (psw, "w"))[nw % 3]
                    ps = pool.tile([128, QTS], f32, tag=tg)
                else:
                    ps = psw.tile([128, QTS], f32, tag="w")
                for kt in range(n_mo):
                    nc.tensor.matmul(
                        ps[:],
                        OT_sb[:, kt, msl],
                        wo_sb[:, kt, nsl],
                        start=(kt == 0),
                        stop=(kt == n_mo - 1),
                    )
                # during drain, alternate evacuation engines so units pipeline
                # instead of serializing behind one engine's queue (GPSIMD
                # cannot read PSUM, so only Act/DVE are eligible)
                if (drain or wo_rot) and nw % 2 == 1:
                    nc.scalar.activation(osb[:, nsl], ps[:], Copy)
                else:
                    nc.vector.tensor_copy(osb[:, nsl], ps[:])
                if nw == 1:
                    nc.sync.dma_start(
                        part_d[b * s + mt * 128 : b * s + (mt + 1) * 128, 0:1024],
                        osb[:, 0:1024],
                    )
                if nw == d // QTS - 1:
                    nc.sync.dma_start(
                        part_d[b * s + mt * 128 : b * s + (mt + 1) * 128, 1024:d],
                        osb[:, 1024:d],
                    )
                    del tiles[("osb", b, mt)]

            def pop_filler(k=1, drain=False):
                for _ in range(k):
                    if wo_queue:
                        b_, mt_, nw_ = wo_queue.popleft()
                        wo_unit(b_, mt_, nw_, drain=drain)

            def queue_wo(b, qt):
                for mt in range(4 * qt, 4 * qt + 4):
                    for nw in range(d // QTS):
                        wo_queue.append((b, mt, nw))

            # ---- attention ----
            def attn_qtile(b, qt, evac_parity):
                mark(f"b{b}_attn{qt}")
                bt = get_batch_tiles(b)
                QT_sb = bt[("QT", b)]
                KT2_sb = bt[("KT2", b)]
                V_sb = bt[("V", b)]
                OT_sb = bt[("OT", b)]
                n_kt = (qt + 1) * (QTS // KTS)  # k tiles needed
                G = n_kt // 2  # strip groups of 2 k-tiles
                qsl = slice(qt * QTS, (qt + 1) * QTS)

                def emit_scores(h, g, sc, e):
                    hb = (h % 2) * 64
                    qh = QT_sb[hb : hb + 64, h // 2, :]
                    kt2 = KT2_sb[hb : hb + 64, :]
                    los = []
                    for j in (0, 1):
                        kt = 2 * g + j
                        o = kt * KTS - qt * QTS
                        lo = max(0, o)
                        los.append(lo)
                        nc.tensor.matmul(
                            sc[:, j, lo:QTS],
                            kt2[:, kt * KTS : (kt + 1) * KTS],
                            qh[:, qt * QTS + lo : (qt + 1) * QTS],
                            start=True,
                            stop=True,
                        )
                    # exp (clipped); diagonal windows get 0/1 lower-tri mask
                    if los[0] == 0 and los[1] == 0 and 2 * g + 1 < 4 * qt:
                        nc.scalar.activation(e[:, :, :], sc[:, :, :], Exp)
                    else:
                        for j in (0, 1):
                            nc.scalar.activation(
                                e[:, j, los[j] : QTS], sc[:, j, los[j] : QTS], Exp
                            )
                    for j in (0, 1):
                        kt = 2 * g + j
                        o = kt * KTS - qt * QTS
                        if o >= 0:
                            eng = nc.vector if (mask_split and j == 0) else nc.gpsimd
                            eng.tensor_tensor(
                                e[:, j, o : o + KTS],
                                e[:, j, o : o + KTS],
                                tri_sb[:],
                                mult_op,
                            )
                    return los

                def emit_pv(h, g, e, los, ops):
                    for j in (0, 1):
                        kt = 2 * g + j
                        lo = los[j]
                        nc.tensor.matmul(
                            ops[:, lo:QTS],
                            V_sb[:, kt, :],
                            e[:, j, lo:QTS],
                            start=(kt == 0),
                            stop=(kt == n_kt - 1),
                        )

                for pair in (0, 1):
                    heads = (2 * pair, 2 * pair + 1)
                    ops = {}
                    pend = {}  # (h, g) -> (e, los) awaiting PV
                    for h in heads:
                        ops[h] = psops.tile(
                            [128, QTS], f32, tag="ops", name=f"ops{h}"
                        )
                    for g in range(G + lag):
                        for h in heads:
                            if g < G:
                                sc = pssc.tile([128, 2, QTS], f32, tag="sc")
                                e = epool.tile([128, 2, QTS], bf16, tag="e")
                                los = emit_scores(h, g, sc, e)
                                pend[(h, g)] = (e, los)
                        for h in heads:
                            if g >= lag:
                                e, los = pend.pop((h, g - lag))
                                emit_pv(h, g - lag, e, los, ops[h])
                        pop_filler(1)
                    # normalize: evacuate O^T+sumexp to SBUF, recip, broadcast,
                    # scale into OT (broadcast + scale on Pool, off the DVE
                    # critical path)
                    for h in heads:
                        hb = (h % 2) * 64
                        osum = rpool.tile([72, QTS], f32, tag="osum")
                        nc.vector.tensor_copy(osum[0:65, :], ops[h][0:65, :])
                        rt = rpool.tile([1, QTS], f32, tag="rt")
                        nc.vector.reciprocal(rt[:], osum[64:65, :])
                        bsb = rpool.tile([64, QTS], f32, tag="bsb")
                        nc.gpsimd.partition_broadcast(bsb[:], rt[:])
                        eng = nc.gpsimd if norm_pool else nc.vector
                        eng.tensor_tensor(
                            OT_sb[hb : hb + 64, h // 2, qsl],
                            osum[0:64, :],
                            bsb[:],
                            mult_op,
                        )
                    pop_filler(1)

            # ---------------- schedule ----------------
            mark("x0_load")
            # first chunk split in half so the first projection matmuls can
            # begin while the second half is still in flight; cos/sin for the
            # first token tile split off so RoPE isn't blocked behind the
            # full tables
            xT0 = get_batch_tiles(0)[("xT", 0)]
            nc.sync.dma_start(
                xT0[:, 0:8, 0:QTS],
                xT_d[0, 0:1024, 0:QTS].rearrange("(j p) c -> p j c", p=128),
            )
            nc.sync.dma_start(
                xT0[:, 8:16, 0:QTS],
                xT_d[0, 1024:2048, 0:QTS].rearrange("(j p) c -> p j c", p=128),
            )
            nc.sync.dma_start(cos_sb[:, 0:QTS], cos_d[:, 0:QTS])
            nc.sync.dma_start(sin_sb[:, 0:QTS], sin_d[:, 0:QTS])
            nc.sync.dma_start(
                wqkv_sb[:, :, 0:256],
                wqkv_d[:, 0:256].rearrange("(j p) c -> p j c", p=128),
            )
            nc.sync.dma_start(tri_sb[:], tri_d[:])
            nc.sync.dma_start(cos_sb[:, QTS:s], cos_d[:, QTS:s])
            nc.sync.dma_start(sin_sb[:, QTS:s], sin_d[:, QTS:s])
            for n in range(1, n_qt):
                load_x_chunk(0, n)
            nc.sync.dma_start(
                wo_sb[:, :, :], wo_d[:, :].rearrange("(j p) c -> p j c", p=128)
            )
            # projections run one token tile ahead of attention so the RoPE /
            # V-transpose chains finish during the previous attention block
            for b in (0, 1):
                for n in range(n_qt):
                    kv_proj_block(b, n)
                    q_proj_block(b, n, 0)
                    pop_filler(1)
                    q_proj_block(b, n, 1)
                    if b == 0:
                        load_x_chunk(1, n)
                    pop_filler(1)
                    if n > 0:
                        attn_qtile(b, n - 1, evac_parity=n % 2)
                        queue_wo(b, n - 1)
                attn_qtile(b, n_qt - 1, evac_parity=0)
                queue_wo(b, n_qt - 1)
            mark("drain")
            while wo_queue:
                pop_filler(1, drain=True)
    mark("end")
    nc.compile()
    return nc


# ---------------- host-side sharding ----------------

_PERM = np.concatenate([np.arange(0, HD, 2), np.arange(1, HD, 2)])  # evens, odds


def make_core_inputs(x, freqs_cos, freqs_sin, wq, wk, wv, wo, s=S, d=D):
    """Build per-core input maps (list of dicts, one per core)."""
    xT = np.ascontiguousarray(np.transpose(x, (0, 2, 1))).astype(BF16)  # [B, D, S]

    cosT = np.ascontiguousarray(freqs_cos.T)  # [32, S]
    sinT = np.ascontiguousarray(freqs_sin.T)
    cosb = np.tile(np.concatenate([cosT, cosT], axis=0), (2, 1)).astype(BF16)  # [128,S]
    sinb = np.tile(np.concatenate([-sinT, sinT], axis=0), (2, 1)).astype(BF16)

    p = np.arange(128)[:, None]
    c = np.arange(128)[None, :]
    tri128 = np.where(c >= p, 1.0, 0.0).astype(BF16)

    scale = 1.0 / math.sqrt(HD)
    in_maps = []
    for cidx in range(N_CORES):
        wq_c = np.concatenate(
            [
                wq[:, (4 * cidx + h) * HD : (4 * cidx + h + 1) * HD][:, _PERM]
                for h in range(HQ)
            ],
            axis=1,
        ) * scale
        wk_c = wk[:, cidx * HD : (cidx + 1) * HD][:, _PERM]
        wv_c = wv[:, cidx * HD : (cidx + 1) * HD]
        wqkv = np.concatenate([wq_c, wk_c, wv_c], axis=1).astype(BF16)  # [D, 384]
        wo_c = np.ascontiguousarray(
            wo[4 * cidx * HD : (4 * cidx + HQ) * HD, :]
        ).astype(BF16)  # [256, D] — O is in original d-order (V unpermuted)
        in_maps.append(
            {
                "xT": xT,
                "wqkv": wqkv,
                "wo_s": wo_c,
                "cosb": cosb,
                "sinb": sinb,
                "tri128": tri128,
            }
        )
    return in_maps


_NC_CACHE = {}


def kernel(x, freqs_cos, freqs_sin, wq, wk, wv, wo):
    from concourse.bass_utils import run_bass_kernel_spmd

    x = np.asarray(x, np.float32)
    freqs_cos = np.asarray(freqs_cos, np.float32)
    freqs_sin = np.asarray(freqs_sin, np.float32)
    wq = np.asarray(wq, np.float32)
    wk = np.asarray(wk, np.float32)
    wv = np.asarray(wv, np.float32)
    wo = np.asarray(wo, np.float32)

    if "nc" not in _NC_CACHE:
        _NC_CACHE["nc"] = build_program()
    nc = _NC_CACHE["nc"]

    in_maps = make_core_inputs(x, freqs_cos, freqs_sin, wq, wk, wv, wo)
    res = run_bass_kernel_spmd(nc, in_maps, list(range(N_CORES)))
    acc = np.zeros((B * S, D), np.float32)
    for r in res.results:
        acc += np.asarray(r["part"], np.float32)
    return acc.reshape(B, S, D).astype(BF16)


# revision 37
# speedup vs baseline: 1.0050x; 1.0050x over previous
"""Trainium2 Bass kernel for nn_Attention_78151224918608.

Dense transformer attention block: QKV proj + RoPE + GQA causal attention
+ output proj. Sharding: tensor-parallel over heads across 8 cores
(core c: Q heads 4c..4c+3, KV head c). Each core computes a partial
output (its heads through wo rows); host sums the 8 bf16 partials in
fp32 and casts to bf16.

Layout strategy (per core, per batch):
  - All matmul operands bf16; accumulation fp32 in PSUM.
  - Projections computed transposed: QKV^T[384, S] = wqkv^T @ x^T so that
    Q^T/K^T (head-dim on partitions) feed the scores matmul directly.
  - RoPE: even/odd pair interleave is folded into wq/wk/wo columns on the
    host (perm = evens-then-odds), turning the pair swap into a 32-row
    block swap done with cross-partition copies on DVE.
  - Scores computed transposed per (b,h): S^T[k,q] = K^T.T @ Q^T, so the
    softmax denominator and P@V both contract over k = partitions:
    PV lhsT = [V | ones-col] gives O^T rows 0:64 and sumexp in row 64.
  - Causal: scores/exp/PV matmuls are column-clipped to the staircase;
    diagonal 128x128 windows get a 0/1 lower-tri multiply after exp.
  - Schedule: x is DMA-streamed n-major (token-tile chunks of all 16
    d-tiles); per token tile: KV proj -> Q proj -> attention for that
    q-tile, with output-projection (wo) work for the previous q-tile
    interleaved between attention pipeline steps to keep PE fed while
    the Activation engine runs exp. b1's x load and projections overlap
    b0's attention (KT2/VT/V/OT tiles double-buffered).
"""

import sys

sys.path.insert(0, "/opt/trn_rl_repo")

import math
from collections import deque
import numpy as np
import ml_dtypes

BF16 = ml_dtypes.bfloat16

# Problem constants (hardcoded per contract).
B = 2
S = 2048
D = 2048
N_HEADS = 32
N_KV_HEADS = 8
HD = 64
N_CORES = 8
HQ = N_HEADS // N_CORES  # 4 q heads per core
M_PROJ = HQ * HD + 2 * HD  # 384: [Q0 Q1 Q2 Q3 | K | V]
QTS = 512  # q tile size (free dim)
KTS = 128  # k tile size (partitions)


def build_program(
    s=S,
    d=D,
    phase_log=None,
    lag=1,
    wo_rot=False,
    norm_pool=False,
    rope_evac_dve=False,
    mask_split=True,
    vt_evac_dve=False,
    osum_act=False,
):
    import concourse.bass as bass
    import concourse.mybir as mybir
    import concourse.tile as tile
    from concourse import bacc

    def mark(label):
        if phase_log is not None:
            phase_log.append((label, len(nc.inst_map)))

    f32 = mybir.dt.float32
    bf16 = mybir.dt.bfloat16
    Exp = mybir.ActivationFunctionType.Exp
    Copy = mybir.ActivationFunctionType.Copy
    add_op = mybir.AluOpType.add
    mult_op = mybir.AluOpType.mult

    n_qt = s // QTS  # q tiles per batch (4)
    n_dkt = d // 128  # contraction tiles for projections (16)
    n_skt = s // KTS  # k tiles per batch (16)
    n_mo = (HQ * HD) // 128  # wo contraction tiles (2)

    nc = bacc.Bacc("TRN2", num_devices=N_CORES)
    xT_d = nc.declare_dram_parameter("xT", [B, d, s], bf16, isOutput=False)
    # weights pre-arranged host-side into SBUF layout [partition, kt, cols]
    # so the loads are single contiguous-per-partition DMA descriptors
    wkv_d = nc.declare_dram_parameter("wkv_r", [128, n_dkt * 128], bf16, isOutput=False)
    wq_d = nc.declare_dram_parameter("wq_r", [128, n_dkt * 256], bf16, isOutput=False)
    wo_d = nc.declare_dram_parameter("wo_r", [128, n_mo * d], bf16, isOutput=False)
    cos_d = nc.declare_dram_parameter("cosb", [128, s], bf16, isOutput=False)
    sin_d = nc.declare_dram_parameter("sinb", [128, s], bf16, isOutput=False)
    tri_d = nc.declare_dram_parameter("tri128", [128, 128], bf16, isOutput=False)
    part_d = nc.declare_dram_parameter("part", [B * s, d], bf16, isOutput=True)

    with tile.TileContext(nc) as tc:
        with (
            tc.tile_pool(name="const", bufs=1) as cpool,
            tc.tile_pool(name="big", bufs=1) as bpool,
            tc.tile_pool(name="work", bufs=3) as wpool,
            tc.tile_pool(name="estrip", bufs=8) as epool,
            tc.tile_pool(name="outp", bufs=4) as opool,
            tc.tile_pool(name="norm", bufs=3) as rpool,
            tc.tile_pool(name="pssc", bufs=2, space="PSUM") as pssc,
            tc.tile_pool(name="psops", bufs=2, space="PSUM") as psops,
            tc.tile_pool(name="psw", bufs=2, space="PSUM") as psw,
        ):
            # ---- constants / weights ----
            cos_sb = cpool.tile([128, s], bf16)
            sin_sb = cpool.tile([128, s], bf16)
            tri_sb = cpool.tile([128, 128], bf16)
            wkv_sb = cpool.tile([128, n_dkt, 128], bf16)
            wq_sb = cpool.tile([128, n_dkt, 256], bf16)
            wo_sb = cpool.tile([128, n_mo, d], bf16)

            # K/V weight columns first so the first projection can start as
            # soon as the first x chunk lands; Q columns + wo arrive behind it.
            nc.sync.dma_start(wkv_sb[:, :, :], wkv_d[:, :])

            tiles = {}

            def get_batch_tiles(b):
                if ("xT", b) not in tiles:
                    tiles[("xT", b)] = bpool.tile(
                        [128, n_dkt, s], bf16, tag="xT", name=f"xT{b}"
                    )
                    tiles[("QT", b)] = bpool.tile(
                        [128, n_mo, s], bf16, tag="QT", name=f"QT{b}"
                    )
                    tiles[("KT2", b)] = bpool.tile(
                        [128, s], bf16, tag="KT2", bufs=2, name=f"KT2{b}"
                    )
                    tiles[("VT", b)] = bpool.tile(
                        [128, s], bf16, tag="VT", bufs=2, name=f"VT{b}"
                    )
                    tiles[("V", b)] = bpool.tile(
                        [128, n_skt, 128], bf16, tag="V", bufs=2, name=f"V{b}"
                    )
                    tiles[("OT", b)] = bpool.tile(
                        [128, n_mo, s], bf16, tag="OT", bufs=2, name=f"OT{b}"
                    )
                return tiles

            def load_x_chunk(b, n):
                """DMA one token-tile chunk of x^T: all d-tiles, cols nsl."""
                xT_sb = get_batch_tiles(b)[("xT", b)]
                nsl = slice(n * QTS, (n + 1) * QTS)
                nc.sync.dma_start(
                    xT_sb[:, :, nsl],
                    xT_d[b, :, nsl].rearrange("(j p) c -> p j c", p=128),
                )

            def rope_pair(dst, ps_src, rows, nsl, swaps):
                """RoPE on `rows` partitions of a psum tile into dst cols nsl."""
                r = slice(0, rows)
                q_raw = wpool.tile([128, QTS], bf16, tag="qraw")
                if rope_evac_dve:
                    nc.vector.tensor_copy(q_raw[r, :], ps_src[r, :])
                else:
                    nc.scalar.activation(q_raw[r, :], ps_src[r, :], Copy)
                t1 = wpool.tile([128, QTS], bf16, tag="t1")
                t2 = wpool.tile([128, QTS], bf16, tag="t2")
                nc.vector.tensor_tensor(t1[r, :], q_raw[r, :], cos_sb[r, nsl], mult_op)
                qsw = wpool.tile([128, QTS], bf16, tag="qsw")
                for r0, r1 in swaps:
                    nc.vector.tensor_copy(qsw[r0 : r0 + 32, :], q_raw[r1 : r1 + 32, :])
                nc.vector.tensor_tensor(t2[r, :], qsw[r, :], sin_sb[r, nsl], mult_op)
                nc.vector.tensor_tensor(dst, t1[r, :], t2[r, :], add_op)

            def kv_proj_block(b, n):
                """K/V projection for token tile n: 16 matmuls + evac."""
                mark(f"b{b}n{n}_kv")
                bt = get_batch_tiles(b)
                xT_sb = bt[("xT", b)]
                KT2_sb = bt[("KT2", b)]
                VT_sb = bt[("VT", b)]
                V_sb = bt[("V", b)]
                nsl = slice(n * QTS, (n + 1) * QTS)
                if n == 0:
                    # ones column / zero pad for PV lhsT
                    nc.gpsimd.memset(V_sb[:, :, 64:128], 0.0)
                    nc.gpsimd.memset(V_sb[:, :, 64:65], 1.0)
                ps = psw.tile([128, QTS], f32, tag="w")
                for kt in range(n_dkt):
                    nc.tensor.matmul(
                        ps[:],
                        wkv_sb[:, kt, :],
                        xT_sb[:, kt, nsl],
                        start=(kt == 0),
                        stop=(kt == n_dkt - 1),
                    )
                # rows 0:64 = K^T (rope), rows 64:128 = V^T (copy)
                rope_pair(KT2_sb[0:64, nsl], ps, 64, nsl, ((0, 32), (32, 0)))
                # duplicate K^T into partitions 64:128 (row-group packing)
                nc.vector.tensor_copy(KT2_sb[64:128, nsl], KT2_sb[0:64, nsl])
                # V^T: plain cast copy into partitions 64:128
                if vt_evac_dve:
                    nc.vector.tensor_copy(VT_sb[64:128, nsl], ps[64:128, :])
                else:
                    nc.scalar.activation(VT_sb[64:128, nsl], ps[64:128, :], Copy)
                # V^T -> V (token-major) via DMA transpose
                for kt in range(n * 4, n * 4 + 4):
                    nc.sync.dma_start_transpose(
                        V_sb[:, kt, 0:64],
                        VT_sb[64:128, kt * KTS : (kt + 1) * KTS],
                    )

            def q_proj_block(b, n, m):
                """Q projection for head pair m (heads 2m, 2m+1), token tile n."""
                bt = get_batch_tiles(b)
                xT_sb = bt[("xT", b)]
                QT_sb = bt[("QT", b)]
                nsl = slice(n * QTS, (n + 1) * QTS)
                ps = psw.tile([128, QTS], f32, tag="w")
                for kt in range(n_dkt):
                    nc.tensor.matmul(
                        ps[:],
                        wq_sb[:, kt, m * 128 : (m + 1) * 128],
                        xT_sb[:, kt, nsl],
                        start=(kt == 0),
                        stop=(kt == n_dkt - 1),
                    )
                rope_pair(
                    QT_sb[:, m, nsl], ps, 128, nsl, ((0, 32), (32, 0), (64, 96), (96, 64))
                )

            # ---- wo filler machinery ----
            wo_queue = deque()

            def wo_unit(b, mt, nw, drain=False):
                """One wo output tile [128 tokens, 512 d-cols]."""
                OT_sb = tiles[("OT", b)]
                msl = slice(mt * 128, (mt + 1) * 128)
                nsl = slice(nw * QTS, (nw + 1) * QTS)
                osb = tiles.get(("osb", b, mt))
                if osb is None:
                    osb = opool.tile([128, d], bf16, tag="osb", name=f"osb{b}_{mt}")
                    tiles[("osb", b, mt)] = osb
                if drain:
                    # attention PSUM pools are idle during the final drain;
                    # borrow them so more units can be in flight
                    pool, tg = ((pssc, "sc"), (psops, "ops"), (psw, "w"))[nw % 3]
                    ps = pool.tile([128, QTS], f32, tag=tg)
                else:
                    ps = psw.tile([128, QTS], f32, tag="w")
                for kt in range(n_mo):
                    nc.tensor.matmul(
                        ps[:],
                        OT_sb[:, kt, msl],
                        wo_sb[:, kt, nsl],
                        start=(kt == 0),
                        stop=(kt == n_mo - 1),
                    )
                # during drain, alternate evacuation engines so units pipeline
                # instead of serializing behind one engine's queue (GPSIMD
                # cannot read PSUM, so only Act/DVE are eligible)
                if (drain or wo_rot) and nw % 2 == 1:
                    nc.scalar.activation(osb[:, nsl], ps[:], Copy)
                else:
                    nc.vector.tensor_copy(osb[:, nsl], ps[:])
                if nw == 1:
                    nc.sync.dma_start(
                        part_d[b * s + mt * 128 : b * s + (mt + 1) * 128, 0:1024],
                        osb[:, 0:1024],
                    )
                if nw == d // QTS - 1:
                    nc.sync.dma_start(
                        part_d[b * s + mt * 128 : b * s + (mt + 1) * 128, 1024:d],
                        osb[:, 1024:d],
                    )
                    del tiles[("osb", b, mt)]

            def pop_filler(k=1, drain=False):
                for _ in range(k):
                    if wo_queue:
                        b_, mt_, nw_ = wo_queue.popleft()
                        wo_unit(b_, mt_, nw_, drain=drain)

            def queue_wo(b, qt):
                for mt in range(4 * qt, 4 * qt + 4):
                    for nw in range(d // QTS):
                        wo_queue.append((b, mt, nw))

            # ---- attention ----
            def attn_qtile(b, qt, evac_parity):
                mark(f"b{b}_attn{qt}")
                bt = get_batch_tiles(b)
                QT_sb = bt[("QT", b)]
                KT2_sb = bt[("KT2", b)]
                V_sb = bt[("V", b)]
                OT_sb = bt[("OT", b)]
                n_kt = (qt + 1) * (QTS // KTS)  # k tiles needed
                G = n_kt // 2  # strip groups of 2 k-tiles
                qsl = slice(qt * QTS, (qt + 1) * QTS)

                def emit_scores(h, g, sc, e):
                    hb = (h % 2) * 64
                    qh = QT_sb[hb : hb + 64, h // 2, :]
                    kt2 = KT2_sb[hb : hb + 64, :]
                    los = []
                    for j in (0, 1):
                        kt = 2 * g + j
                        o = kt * KTS - qt * QTS
                        lo = max(0, o)
                        los.append(lo)
                        nc.tensor.matmul(
                            sc[:, j, lo:QTS],
                            kt2[:, kt * KTS : (kt + 1) * KTS],
                            qh[:, qt * QTS + lo : (qt + 1) * QTS],
                            start=True,
                            stop=True,
                        )
                    # exp (clipped); diagonal windows get 0/1 lower-tri mask
                    if los[0] == 0 and los[1] == 0 and 2 * g + 1 < 4 * qt:
                        nc.scalar.activation(e[:, :, :], sc[:, :, :], Exp)
                    else:
                        for j in (0, 1):
                            nc.scalar.activation(
                                e[:, j, los[j] : QTS], sc[:, j, los[j] : QTS], Exp
                            )
                    for j in (0, 1):
                        kt = 2 * g + j
                        o = kt * KTS - qt * QTS
                        if o >= 0:
                            eng = nc.vector if (mask_split and j == 0) else nc.gpsimd
                            eng.tensor_tensor(
                                e[:, j, o : o + KTS],
                                e[:, j, o : o + KTS],
                                tri_sb[:],
                                mult_op,
                            )
                    return los

                def emit_pv(h, g, e, los, ops):
                    for j in (0, 1):
                        kt = 2 * g + j
                        lo = los[j]
                        nc.tensor.matmul(
                            ops[:, lo:QTS],
                            V_sb[:, kt, :],
                            e[:, j, lo:QTS],
                            start=(kt == 0),
                            stop=(kt == n_kt - 1),
                        )

                for pair in (0, 1):
                    heads = (2 * pair, 2 * pair + 1)
                    ops = {}
                    pend = {}  # (h, g) -> (e, los) awaiting PV
                    for h in heads:
                        ops[h] = psops.tile(
                            [128, QTS], f32, tag="ops", name=f"ops{h}"
                        )
                    for g in range(G + lag):
                        for h in heads:
                            if g < G:
                                sc = pssc.tile([128, 2, QTS], f32, tag="sc")
                                e = epool.tile([128, 2, QTS], bf16, tag="e")
                                los = emit_scores(h, g, sc, e)
                                pend[(h, g)] = (e, los)
                        for h in heads:
                            if g >= lag:
                                e, los = pend.pop((h, g - lag))
                                emit_pv(h, g - lag, e, los, ops[h])
                        pop_filler(1)
                    # normalize: evacuate O^T+sumexp to SBUF, recip, broadcast,
                    # scale into OT (broadcast + scale on Pool, off the DVE
                    # critical path)
                    for h in heads:
                        hb = (h % 2) * 64
                        osum = rpool.tile([72, QTS], f32, tag="osum")
                        if osum_act:
                            nc.scalar.activation(osum[0:65, :], ops[h][0:65, :], Copy)
                        else:
                            nc.vector.tensor_copy(osum[0:65, :], ops[h][0:65, :])
                        rt = rpool.tile([1, QTS], f32, tag="rt")
                        nc.vector.reciprocal(rt[:], osum[64:65, :])
                        bsb = rpool.tile([64, QTS], f32, tag="bsb")
                        nc.gpsimd.partition_broadcast(bsb[:], rt[:])
                        eng = nc.gpsimd if norm_pool else nc.vector
                        eng.tensor_tensor(
                            OT_sb[hb : hb + 64, h // 2, qsl],
                            osum[0:64, :],
                            bsb[:],
                            mult_op,
                        )
                    pop_filler(1)

            # ---------------- schedule ----------------
            mark("x0_load")
            # first chunk split in half so the first projection matmuls can
            # begin while the second half is still in flight; cos/sin for the
            # first token tile split off so RoPE isn't blocked behind the
            # full tables
            xT0 = get_batch_tiles(0)[("xT", 0)]
            nc.sync.dma_start(
                xT0[:, 0:8, 0:QTS],
                xT_d[0, 0:1024, 0:QTS].rearrange("(j p) c -> p j c", p=128),
            )
            nc.sync.dma_start(
                xT0[:, 8:16, 0:QTS],
                xT_d[0, 1024:2048, 0:QTS].rearrange("(j p) c -> p j c", p=128),
            )
            nc.sync.dma_start(cos_sb[:, 0:QTS], cos_d[:, 0:QTS])
            nc.sync.dma_start(sin_sb[:, 0:QTS], sin_d[:, 0:QTS])
            nc.sync.dma_start(wq_sb[:, :, :], wq_d[:, :])
            nc.sync.dma_start(tri_sb[:], tri_d[:])
            nc.sync.dma_start(cos_sb[:, QTS:s], cos_d[:, QTS:s])
            nc.sync.dma_start(sin_sb[:, QTS:s], sin_d[:, QTS:s])
            for n in range(1, n_qt):
                load_x_chunk(0, n)
            nc.sync.dma_start(wo_sb[:, :, :], wo_d[:, :])
            # projections run one token tile ahead of attention so the RoPE /
            # V-transpose chains finish during the previous attention block
            for b in (0, 1):
                for n in range(n_qt):
                    kv_proj_block(b, n)
                    q_proj_block(b, n, 0)
                    pop_filler(1)
                    q_proj_block(b, n, 1)
                    if b == 0:
                        load_x_chunk(1, n)
                    pop_filler(1)
                    if n > 0:
                        attn_qtile(b, n - 1, evac_parity=n % 2)
                        queue_wo(b, n - 1)
                attn_qtile(b, n_qt - 1, evac_parity=0)
                queue_wo(b, n_qt - 1)
            mark("drain")
            while wo_queue:
                pop_filler(1, drain=True)
    mark("end")
    nc.compile()
    return nc


# ---------------- host-side sharding ----------------

_PERM = np.concatenate([np.arange(0, HD, 2), np.arange(1, HD, 2)])  # evens, odds


def make_core_inputs(x, freqs_cos, freqs_sin, wq, wk, wv, wo, s=S, d=D):
    """Build per-core input maps (list of dicts, one per core)."""
    xT = np.ascontiguousarray(np.transpose(x, (0, 2, 1))).astype(BF16)  # [B, D, S]

    cosT = np.ascontiguousarray(freqs_cos.T)  # [32, S]
    sinT = np.ascontiguousarray(freqs_sin.T)
    cosb = np.tile(np.concatenate([cosT, cosT], axis=0), (2, 1)).astype(BF16)  # [128,S]
    sinb = np.tile(np.concatenate([-sinT, sinT], axis=0), (2, 1)).astype(BF16)

    p = np.arange(128)[:, None]
    c = np.arange(128)[None, :]
    tri128 = np.where(c >= p, 1.0, 0.0).astype(BF16)

    scale = 1.0 / math.sqrt(HD)
    in_maps = []
    for cidx in range(N_CORES):
        wq_c = np.concatenate(
            [
                wq[:, (4 * cidx + h) * HD : (4 * cidx + h + 1) * HD][:, _PERM]
                for h in range(HQ)
            ],
            axis=1,
        ) * scale
        wk_c = wk[:, cidx * HD : (cidx + 1) * HD][:, _PERM]
        wv_c = wv[:, cidx * HD : (cidx + 1) * HD]
        wkv_c = np.concatenate([wk_c, wv_c], axis=1)  # [D, 128]
        wo_c = wo[4 * cidx * HD : (4 * cidx + HQ) * HD, :]  # [256, D]
        # rearrange into SBUF layout [partition, kt*cols] so each load is one
        # contiguous-per-partition DMA
        wkv_r = np.ascontiguousarray(
            wkv_c.reshape(16, 128, 128).transpose(1, 0, 2).reshape(128, -1)
        ).astype(BF16)
        wq_r = np.ascontiguousarray(
            wq_c.reshape(16, 128, 256).transpose(1, 0, 2).reshape(128, -1)
        ).astype(BF16)
        wo_r = np.ascontiguousarray(
            wo_c.reshape(2, 128, D).transpose(1, 0, 2).reshape(128, -1)
        ).astype(BF16)
        in_maps.append(
            {
                "xT": xT,
                "wkv_r": wkv_r,
                "wq_r": wq_r,
                "wo_r": wo_r,
                "cosb": cosb,
                "sinb": sinb,
                "tri128": tri128,
            }
        )
    return in_maps


_NC_CACHE = {}


def kernel(x, freqs_cos, freqs_sin, wq, wk, wv, wo):
    from concourse.bass_utils import run_bass_kernel_spmd

    x = np.asarray(x, np.float32)
    freqs_cos = np.asarray(freqs_cos, np.float32)
    freqs_sin = np.asarray(freqs_sin, np.float32)
    wq = np.asarray(wq, np.float32)
    wk = np.asarray(wk, np.float32)
    wv = np.asarray(wv, np.float32)
    wo = np.asarray(wo, np.float32)

    if "nc" not in _NC_CACHE:
        _NC_CACHE["nc"] = build_program()
    nc = _NC_CACHE["nc"]

    in_maps = make_core_inputs(x, freqs_cos, freqs_sin, wq, wk, wv, wo)
    res = run_bass_kernel_spmd(nc, in_maps, list(range(N_CORES)))
    acc = np.zeros((B * S, D), np.float32)
    for r in res.results:
        acc += np.asarray(r["part"], np.float32)
    return acc.reshape(B, S, D).astype(BF16)


# revision 41
# speedup vs baseline: 1.0069x; 1.0020x over previous
"""Trainium2 Bass kernel for nn_Attention_78151224918608.

Dense transformer attention block: QKV proj + RoPE + GQA causal attention
+ output proj. Sharding: tensor-parallel over heads across 8 cores
(core c: Q heads 4c..4c+3, KV head c). Each core computes a partial
output (its heads through wo rows); host sums the 8 bf16 partials in
fp32 and casts to bf16.

Layout strategy (per core, per batch):
  - All matmul operands bf16; accumulation fp32 in PSUM.
  - Projections computed transposed: QKV^T[384, S] = wqkv^T @ x^T so that
    Q^T/K^T (head-dim on partitions) feed the scores matmul directly.
  - RoPE: even/odd pair interleave is folded into wq/wk/wo columns on the
    host (perm = evens-then-odds), turning the pair swap into a 32-row
    block swap done with cross-partition copies on DVE.
  - Scores computed transposed per (b,h): S^T[k,q] = K^T.T @ Q^T, so the
    softmax denominator and P@V both contract over k = partitions:
    PV lhsT = [V | ones-col] gives O^T rows 0:64 and sumexp in row 64.
  - Causal: scores/exp/PV matmuls are column-clipped to the staircase;
    diagonal 128x128 windows get a 0/1 lower-tri multiply after exp.
  - Schedule: x is DMA-streamed n-major (token-tile chunks of all 16
    d-tiles); per token tile: KV proj -> Q proj -> attention for that
    q-tile, with output-projection (wo) work for the previous q-tile
    interleaved between attention pipeline steps to keep PE fed while
    the Activation engine runs exp. b1's x load and projections overlap
    b0's attention (KT2/VT/V/OT tiles double-buffered).
"""

import sys

sys.path.insert(0, "/opt/trn_rl_repo")

import math
from collections import deque
import numpy as np
import ml_dtypes

BF16 = ml_dtypes.bfloat16

# Problem constants (hardcoded per contract).
B = 2
S = 2048
D = 2048
N_HEADS = 32
N_KV_HEADS = 8
HD = 64
N_CORES = 8
HQ = N_HEADS // N_CORES  # 4 q heads per core
M_PROJ = HQ * HD + 2 * HD  # 384: [Q0 Q1 Q2 Q3 | K | V]
QTS = 512  # q tile size (free dim)
KTS = 128  # k tile size (partitions)


def build_program(
    s=S,
    d=D,
    phase_log=None,
    lag=1,
    wo_rot=False,
    norm_pool=False,
    rope_evac_dve=False,
    mask_split=True,
    vt_evac_dve=False,
    osum_act=False,
):
    import concourse.bass as bass
    import concourse.mybir as mybir
    import concourse.tile as tile
    from concourse import bacc

    def mark(label):
        if phase_log is not None:
            phase_log.append((label, len(nc.inst_map)))

    f32 = mybir.dt.float32
    bf16 = mybir.dt.bfloat16
    Exp = mybir.ActivationFunctionType.Exp
    Copy = mybir.ActivationFunctionType.Copy
    add_op = mybir.AluOpType.add
    mult_op = mybir.AluOpType.mult

    n_qt = s // QTS  # q tiles per batch (4)
    n_dkt = d // 128  # contraction tiles for projections (16)
    n_skt = s // KTS  # k tiles per batch (16)
    n_mo = (HQ * HD) // 128  # wo contraction tiles (2)

    nc = bacc.Bacc("TRN2", num_devices=N_CORES)
    xT_d = nc.declare_dram_parameter("xT", [B, d, s], bf16, isOutput=False)
    # weights pre-arranged host-side into SBUF layout [partition, kt, cols]
    # so the loads are single contiguous-per-partition DMA descriptors
    wkv_d = nc.declare_dram_parameter("wkv_r", [128, n_dkt * 128], bf16, isOutput=False)
    wq_d = nc.declare_dram_parameter("wq_r", [128, n_dkt * 256], bf16, isOutput=False)
    wo_d = nc.declare_dram_parameter("wo_r", [128, n_mo * d], bf16, isOutput=False)
    cos_d = nc.declare_dram_parameter("cosb", [128, s], bf16, isOutput=False)
    sin_d = nc.declare_dram_parameter("sinb", [128, s], bf16, isOutput=False)
    tri_d = nc.declare_dram_parameter("tri128", [128, 128], bf16, isOutput=False)
    part_d = nc.declare_dram_parameter("part", [B * s, d], bf16, isOutput=True)

    with tile.TileContext(nc) as tc:
        with (
            tc.tile_pool(name="const", bufs=1) as cpool,
            tc.tile_pool(name="big", bufs=1) as bpool,
            tc.tile_pool(name="work", bufs=3) as wpool,
            tc.tile_pool(name="estrip", bufs=8) as epool,
            tc.tile_pool(name="outp", bufs=4) as opool,
            tc.tile_pool(name="norm", bufs=3) as rpool,
            tc.tile_pool(name="pssc", bufs=2, space="PSUM") as pssc,
            tc.tile_pool(name="psops", bufs=2, space="PSUM") as psops,
            tc.tile_pool(name="psw", bufs=2, space="PSUM") as psw,
        ):
            # ---- constants / weights ----
            cos_sb = cpool.tile([128, s], bf16)
            sin_sb = cpool.tile([128, s], bf16)
            tri_sb = cpool.tile([128, 128], bf16)
            wkv_sb = cpool.tile([128, n_dkt, 128], bf16)
            wq_sb = cpool.tile([128, n_dkt, 256], bf16)
            wo_sb = cpool.tile([128, n_mo, d], bf16)

            # K/V weight columns first so the first projection can start as
            # soon as the first x chunk lands; Q columns + wo arrive behind it.
            nc.sync.dma_start(wkv_sb[:, :, :], wkv_d[:, :])

            tiles = {}

            def get_batch_tiles(b):
                if ("xT", b) not in tiles:
                    tiles[("xT", b)] = bpool.tile(
                        [128, n_dkt, s], bf16, tag="xT", name=f"xT{b}"
                    )
                    tiles[("QT", b)] = bpool.tile(
                        [128, n_mo, s], bf16, tag="QT", name=f"QT{b}"
                    )
                    tiles[("KT2", b)] = bpool.tile(
                        [128, s], bf16, tag="KT2", bufs=2, name=f"KT2{b}"
                    )
                    tiles[("VT", b)] = bpool.tile(
                        [128, s], bf16, tag="VT", bufs=2, name=f"VT{b}"
                    )
                    tiles[("V", b)] = bpool.tile(
                        [128, n_skt, 128], bf16, tag="V", bufs=2, name=f"V{b}"
                    )
                    tiles[("OT", b)] = bpool.tile(
                        [128, n_mo, s], bf16, tag="OT", bufs=2, name=f"OT{b}"
                    )
                return tiles

            def load_x_chunk(b, n, split=1):
                """DMA one token-tile chunk of x^T: all d-tiles, cols nsl.

                split>1 breaks it into several DMAs so latency-critical
                transfers (V transposes, outputs) don't queue behind one
                long transfer on the DMA engines.
                """
                xT_sb = get_batch_tiles(b)[("xT", b)]
                nsl = slice(n * QTS, (n + 1) * QTS)
                step = n_dkt // split
                for i in range(split):
                    ksl = slice(i * step, (i + 1) * step)
                    nc.sync.dma_start(
                        xT_sb[:, ksl, nsl],
                        xT_d[b, i * step * 128 : (i + 1) * step * 128, nsl].rearrange(
                            "(j p) c -> p j c", p=128
                        ),
                    )

            def rope_pair(dst, ps_src, rows, nsl, swaps):
                """RoPE on `rows` partitions of a psum tile into dst cols nsl."""
                r = slice(0, rows)
                q_raw = wpool.tile([128, QTS], bf16, tag="qraw")
                if rope_evac_dve:
                    nc.vector.tensor_copy(q_raw[r, :], ps_src[r, :])
                else:
                    nc.scalar.activation(q_raw[r, :], ps_src[r, :], Copy)
                t1 = wpool.tile([128, QTS], bf16, tag="t1")
                t2 = wpool.tile([128, QTS], bf16, tag="t2")
                nc.vector.tensor_tensor(t1[r, :], q_raw[r, :], cos_sb[r, nsl], mult_op)
                qsw = wpool.tile([128, QTS], bf16, tag="qsw")
                for r0, r1 in swaps:
                    nc.vector.tensor_copy(qsw[r0 : r0 + 32, :], q_raw[r1 : r1 + 32, :])
                nc.vector.tensor_tensor(t2[r, :], qsw[r, :], sin_sb[r, nsl], mult_op)
                nc.vector.tensor_tensor(dst, t1[r, :], t2[r, :], add_op)

            def kv_proj_block(b, n):
                """K/V projection for token tile n: 16 matmuls + evac."""
                mark(f"b{b}n{n}_kv")
                bt = get_batch_tiles(b)
                xT_sb = bt[("xT", b)]
                KT2_sb = bt[("KT2", b)]
                VT_sb = bt[("VT", b)]
                V_sb = bt[("V", b)]
                nsl = slice(n * QTS, (n + 1) * QTS)
                if n == 0:
                    # ones column / zero pad for PV lhsT
                    nc.gpsimd.memset(V_sb[:, :, 64:128], 0.0)
                    nc.gpsimd.memset(V_sb[:, :, 64:65], 1.0)
                ps = psw.tile([128, QTS], f32, tag="w")
                for kt in range(n_dkt):
                    nc.tensor.matmul(
                        ps[:],
                        wkv_sb[:, kt, :],
                        xT_sb[:, kt, nsl],
                        start=(kt == 0),
                        stop=(kt == n_dkt - 1),
                    )
                # rows 0:64 = K^T (rope), rows 64:128 = V^T (copy)
                rope_pair(KT2_sb[0:64, nsl], ps, 64, nsl, ((0, 32), (32, 0)))
                # duplicate K^T into partitions 64:128 (row-group packing)
                nc.vector.tensor_copy(KT2_sb[64:128, nsl], KT2_sb[0:64, nsl])
                # V^T: plain cast copy into partitions 64:128
                if vt_evac_dve:
                    nc.vector.tensor_copy(VT_sb[64:128, nsl], ps[64:128, :])
                else:
                    nc.scalar.activation(VT_sb[64:128, nsl], ps[64:128, :], Copy)
                # V^T -> V (token-major) via DMA transpose
                for kt in range(n * 4, n * 4 + 4):
                    nc.sync.dma_start_transpose(
                        V_sb[:, kt, 0:64],
                        VT_sb[64:128, kt * KTS : (kt + 1) * KTS],
                    )

            def q_proj_block(b, n, m):
                """Q projection for head pair m (heads 2m, 2m+1), token tile n."""
                bt = get_batch_tiles(b)
                xT_sb = bt[("xT", b)]
                QT_sb = bt[("QT", b)]
                nsl = slice(n * QTS, (n + 1) * QTS)
                ps = psw.tile([128, QTS], f32, tag="w")
                for kt in range(n_dkt):
                    nc.tensor.matmul(
                        ps[:],
                        wq_sb[:, kt, m * 128 : (m + 1) * 128],
                        xT_sb[:, kt, nsl],
                        start=(kt == 0),
                        stop=(kt == n_dkt - 1),
                    )
                rope_pair(
                    QT_sb[:, m, nsl], ps, 128, nsl, ((0, 32), (32, 0), (64, 96), (96, 64))
                )

            # ---- filler machinery: closures emitted between attention steps ----
            wo_queue = deque()
            pre_fillers = deque()  # emitted with priority over wo units

            def wo_unit(b, mt, nw, drain=False):
                """One wo output tile [128 tokens, 512 d-cols]."""
                OT_sb = tiles[("OT", b)]
                msl = slice(mt * 128, (mt + 1) * 128)
                nsl = slice(nw * QTS, (nw + 1) * QTS)
                osb = tiles.get(("osb", b, mt))
                if osb is None:
                    osb = opool.tile([128, d], bf16, tag="osb", name=f"osb{b}_{mt}")
                    tiles[("osb", b, mt)] = osb
                if drain:
                    # attention PSUM pools are idle during the final drain;
                    # borrow them so more units can be in flight
                    pool, tg = ((pssc, "sc"), (psops, "ops"), (psw, "w"))[nw % 3]
                    ps = pool.tile([128, QTS], f32, tag=tg)
                else:
                    ps = psw.tile([128, QTS], f32, tag="w")
                for kt in range(n_mo):
                    nc.tensor.matmul(
                        ps[:],
                        OT_sb[:, kt, msl],
                        wo_sb[:, kt, nsl],
                        start=(kt == 0),
                        stop=(kt == n_mo - 1),
                    )
                # during drain, alternate evacuation engines so units pipeline
                # instead of serializing behind one engine's queue (GPSIMD
                # cannot read PSUM, so only Act/DVE are eligible)
                if (drain or wo_rot) and nw % 2 == 1:
                    nc.scalar.activation(osb[:, nsl], ps[:], Copy)
                else:
                    nc.vector.tensor_copy(osb[:, nsl], ps[:])
                if nw == 1:
                    nc.sync.dma_start(
                        part_d[b * s + mt * 128 : b * s + (mt + 1) * 128, 0:1024],
                        osb[:, 0:1024],
                    )
                if nw == d // QTS - 1:
                    nc.sync.dma_start(
                        part_d[b * s + mt * 128 : b * s + (mt + 1) * 128, 1024:d],
                        osb[:, 1024:d],
                    )
                    del tiles[("osb", b, mt)]

            def pop_filler(k=1, drain=False):
                for _ in range(k):
                    if pre_fillers:
                        pre_fillers.popleft()()
                    elif wo_queue:
                        b_, mt_, nw_ = wo_queue.popleft()
                        wo_unit(b_, mt_, nw_, drain=drain)

            def queue_wo(b, qt):
                for mt in range(4 * qt, 4 * qt + 4):
                    for nw in range(d // QTS):
                        wo_queue.append((b, mt, nw))

            # ---- attention ----
            def attn_qtile(b, qt, evac_parity):
                mark(f"b{b}_attn{qt}")
                bt = get_batch_tiles(b)
                QT_sb = bt[("QT", b)]
                KT2_sb = bt[("KT2", b)]
                V_sb = bt[("V", b)]
                OT_sb = bt[("OT", b)]
                n_kt = (qt + 1) * (QTS // KTS)  # k tiles needed
                G = n_kt // 2  # strip groups of 2 k-tiles
                qsl = slice(qt * QTS, (qt + 1) * QTS)

                def emit_scores(h, g, sc, e):
                    hb = (h % 2) * 64
                    qh = QT_sb[hb : hb + 64, h // 2, :]
                    kt2 = KT2_sb[hb : hb + 64, :]
                    los = []
                    for j in (0, 1):
                        kt = 2 * g + j
                        o = kt * KTS - qt * QTS
                        lo = max(0, o)
                        los.append(lo)
                        nc.tensor.matmul(
                            sc[:, j, lo:QTS],
                            kt2[:, kt * KTS : (kt + 1) * KTS],
                            qh[:, qt * QTS + lo : (qt + 1) * QTS],
                            start=True,
                            stop=True,
                        )
                    # exp (clipped); diagonal windows get 0/1 lower-tri mask
                    if los[0] == 0 and los[1] == 0 and 2 * g + 1 < 4 * qt:
                        nc.scalar.activation(e[:, :, :], sc[:, :, :], Exp)
                    else:
                        for j in (0, 1):
                            nc.scalar.activation(
                                e[:, j, los[j] : QTS], sc[:, j, los[j] : QTS], Exp
                            )
                    for j in (0, 1):
                        kt = 2 * g + j
                        o = kt * KTS - qt * QTS
                        if o >= 0:
                            eng = nc.vector if (mask_split and j == 0) else nc.gpsimd
                            eng.tensor_tensor(
                                e[:, j, o : o + KTS],
                                e[:, j, o : o + KTS],
                                tri_sb[:],
                                mult_op,
                            )
                    return los

                def emit_pv(h, g, e, los, ops):
                    for j in (0, 1):
                        kt = 2 * g + j
                        lo = los[j]
                        nc.tensor.matmul(
                            ops[:, lo:QTS],
                            V_sb[:, kt, :],
                            e[:, j, lo:QTS],
                            start=(kt == 0),
                            stop=(kt == n_kt - 1),
                        )

                for pair in (0, 1):
                    heads = (2 * pair, 2 * pair + 1)
                    ops = {}
                    pend = {}  # (h, g) -> (e, los) awaiting PV
                    for h in heads:
                        ops[h] = psops.tile(
                            [128, QTS], f32, tag="ops", name=f"ops{h}"
                        )
                    for g in range(G + lag):
                        for h in heads:
                            if g < G:
                                sc = pssc.tile([128, 2, QTS], f32, tag="sc")
                                e = epool.tile([128, 2, QTS], bf16, tag="e")
                                los = emit_scores(h, g, sc, e)
                                pend[(h, g)] = (e, los)
                        for h in heads:
                            if g >= lag:
                                e, los = pend.pop((h, g - lag))
                                emit_pv(h, g - lag, e, los, ops[h])
                        pop_filler(1)
                    # normalize: evacuate O^T+sumexp to SBUF, recip, broadcast,
                    # scale into OT (broadcast + scale on Pool, off the DVE
                    # critical path)
                    for h in heads:
                        hb = (h % 2) * 64
                        osum = rpool.tile([72, QTS], f32, tag="osum")
                        if osum_act:
                            nc.scalar.activation(osum[0:65, :], ops[h][0:65, :], Copy)
                        else:
                            nc.vector.tensor_copy(osum[0:65, :], ops[h][0:65, :])
                        rt = rpool.tile([1, QTS], f32, tag="rt")
                        nc.vector.reciprocal(rt[:], osum[64:65, :])
                        bsb = rpool.tile([64, QTS], f32, tag="bsb")
                        nc.gpsimd.partition_broadcast(bsb[:], rt[:])
                        eng = nc.gpsimd if norm_pool else nc.vector
                        eng.tensor_tensor(
                            OT_sb[hb : hb + 64, h // 2, qsl],
                            osum[0:64, :],
                            bsb[:],
                            mult_op,
                        )
                    pop_filler(1)

            # ---------------- schedule ----------------
            mark("x0_load")
            # first chunk split in half so the first projection matmuls can
            # begin while the second half is still in flight; cos/sin for the
            # first token tile split off so RoPE isn't blocked behind the
            # full tables
            xT0 = get_batch_tiles(0)[("xT", 0)]
            nc.sync.dma_start(
                xT0[:, 0:8, 0:QTS],
                xT_d[0, 0:1024, 0:QTS].rearrange("(j p) c -> p j c", p=128),
            )
            nc.sync.dma_start(
                xT0[:, 8:16, 0:QTS],
                xT_d[0, 1024:2048, 0:QTS].rearrange("(j p) c -> p j c", p=128),
            )
            nc.sync.dma_start(cos_sb[:, 0:QTS], cos_d[:, 0:QTS])
            nc.sync.dma_start(sin_sb[:, 0:QTS], sin_d[:, 0:QTS])
            nc.sync.dma_start(wq_sb[:, :, :], wq_d[:, :])
            nc.sync.dma_start(tri_sb[:], tri_d[:])
            nc.sync.dma_start(cos_sb[:, QTS:s], cos_d[:, QTS:s])
            nc.sync.dma_start(sin_sb[:, QTS:s], sin_d[:, QTS:s])
            for n in range(1, n_qt):
                load_x_chunk(0, n)
            nc.sync.dma_start(wo_sb[:, :, :], wo_d[:, :])
            # projections run one token tile ahead of attention so the RoPE /
            # V-transpose chains finish during the previous attention block;
            # b1's first projections interleave into b0's last attention block
            for b in (0, 1):
                for n in range(n_qt):
                    if not (b == 1 and n == 0):
                        kv_proj_block(b, n)
                        q_proj_block(b, n, 0)
                        pop_filler(1)
                        q_proj_block(b, n, 1)
                    if b == 0:
                        load_x_chunk(1, n, split=4)
                    pop_filler(1)
                    if n > 0:
                        attn_qtile(b, n - 1, evac_parity=n % 2)
                        queue_wo(b, n - 1)
                if b == 0:
                    pre_fillers.append(lambda: kv_proj_block(1, 0))
                    pre_fillers.append(lambda: q_proj_block(1, 0, 0))
                    pre_fillers.append(lambda: q_proj_block(1, 0, 1))
                attn_qtile(b, n_qt - 1, evac_parity=0)
                queue_wo(b, n_qt - 1)
                while pre_fillers:
                    pre_fillers.popleft()()
            mark("drain")
            while wo_queue:
                pop_filler(1, drain=True)
    mark("end")
    nc.compile()
    return nc


# ---------------- host-side sharding ----------------

_PERM = np.concatenate([np.arange(0, HD, 2), np.arange(1, HD, 2)])  # evens, odds


def make_core_inputs(x, freqs_cos, freqs_sin, wq, wk, wv, wo, s=S, d=D):
    """Build per-core input maps (list of dicts, one per core)."""
    xT = np.ascontiguousarray(np.transpose(x, (0, 2, 1))).astype(BF16)  # [B, D, S]

    cosT = np.ascontiguousarray(freqs_cos.T)  # [32, S]
    sinT = np.ascontiguousarray(freqs_sin.T)
    cosb = np.tile(np.concatenate([cosT, cosT], axis=0), (2, 1)).astype(BF16)  # [128,S]
    sinb = np.tile(np.concatenate([-sinT, sinT], axis=0), (2, 1)).astype(BF16)

    p = np.arange(128)[:, None]
    c = np.arange(128)[None, :]
    tri128 = np.where(c >= p, 1.0, 0.0).astype(BF16)

    scale = 1.0 / math.sqrt(HD)
    in_maps = []
    for cidx in range(N_CORES):
        wq_c = np.concatenate(
            [
                wq[:, (4 * cidx + h) * HD : (4 * cidx + h + 1) * HD][:, _PERM]
                for h in range(HQ)
            ],
            axis=1,
        ) * scale
        wk_c = wk[:, cidx * HD : (cidx + 1) * HD][:, _PERM]
        wv_c = wv[:, cidx * HD : (cidx + 1) * HD]
        wkv_c = np.concatenate([wk_c, wv_c], axis=1)  # [D, 128]
        wo_c = wo[4 * cidx * HD : (4 * cidx + HQ) * HD, :]  # [256, D]
        # rearrange into SBUF layout [partition, kt*cols] so each load is one
        # contiguous-per-partition DMA
        wkv_r = np.ascontiguousarray(
            wkv_c.reshape(16, 128, 128).transpose(1, 0, 2).reshape(128, -1)
        ).astype(BF16)
        wq_r = np.ascontiguousarray(
            wq_c.reshape(16, 128, 256).transpose(1, 0, 2).reshape(128, -1)
        ).astype(BF16)
        wo_r = np.ascontiguousarray(
            wo_c.reshape(2, 128, D).transpose(1, 0, 2).reshape(128, -1)
        ).astype(BF16)
        in_maps.append(
            {
                "xT": xT,
                "wkv_r": wkv_r,
                "wq_r": wq_r,
                "wo_r": wo_r,
                "cosb": cosb,
                "sinb": sinb,
                "tri128": tri128,
            }
        )
    return in_maps


_NC_CACHE = {}


def kernel(x, freqs_cos, freqs_sin, wq, wk, wv, wo):
    from concourse.bass_utils import run_bass_kernel_spmd

    x = np.asarray(x, np.float32)
    freqs_cos = np.asarray(freqs_cos, np.float32)
    freqs_sin = np.asarray(freqs_sin, np.float32)
    wq = np.asarray(wq, np.float32)
    wk = np.asarray(wk, np.float32)
    wv = np.asarray(wv, np.float32)
    wo = np.asarray(wo, np.float32)

    if "nc" not in _NC_CACHE:
        _NC_CACHE["nc"] = build_program()
    nc = _NC_CACHE["nc"]

    in_maps = make_core_inputs(x, freqs_cos, freqs_sin, wq, wk, wv, wo)
    res = run_bass_kernel_spmd(nc, in_maps, list(range(N_CORES)))
    acc = np.zeros((B * S, D), np.float32)
    for r in res.results:
        acc += np.asarray(r["part"], np.float32)
    return acc.reshape(B, S, D).astype(BF16)


# revision 42
# speedup vs baseline: 1.0141x; 1.0071x over previous
"""Trainium2 Bass kernel for nn_Attention_78151224918608.

Dense transformer attention block: QKV proj + RoPE + GQA causal attention
+ output proj. Sharding: tensor-parallel over heads across 8 cores
(core c: Q heads 4c..4c+3, KV head c). Each core computes a partial
output (its heads through wo rows); host sums the 8 bf16 partials in
fp32 and casts to bf16.

Layout strategy (per core, per batch):
  - All matmul operands bf16; accumulation fp32 in PSUM.
  - Projections computed transposed: QKV^T[384, S] = wqkv^T @ x^T so that
    Q^T/K^T (head-dim on partitions) feed the scores matmul directly.
  - RoPE: even/odd pair interleave is folded into wq/wk/wo columns on the
    host (perm = evens-then-odds), turning the pair swap into a 32-row
    block swap done with cross-partition copies on DVE.
  - Scores computed transposed per (b,h): S^T[k,q] = K^T.T @ Q^T, so the
    softmax denominator and P@V both contract over k = partitions:
    PV lhsT = [V | ones-col] gives O^T rows 0:64 and sumexp in row 64.
  - Causal: scores/exp/PV matmuls are column-clipped to the staircase;
    diagonal 128x128 windows get a 0/1 lower-tri multiply after exp.
  - Schedule: x is DMA-streamed n-major (token-tile chunks of all 16
    d-tiles); per token tile: KV proj -> Q proj -> attention for that
    q-tile, with output-projection (wo) work for the previous q-tile
    interleaved between attention pipeline steps to keep PE fed while
    the Activation engine runs exp. b1's x load and projections overlap
    b0's attention (KT2/VT/V/OT tiles double-buffered).
"""

import sys

sys.path.insert(0, "/opt/trn_rl_repo")

import math
from collections import deque
import numpy as np
import ml_dtypes

BF16 = ml_dtypes.bfloat16

# Problem constants (hardcoded per contract).
B = 2
S = 2048
D = 2048
N_HEADS = 32
N_KV_HEADS = 8
HD = 64
N_CORES = 8
HQ = N_HEADS // N_CORES  # 4 q heads per core
M_PROJ = HQ * HD + 2 * HD  # 384: [Q0 Q1 Q2 Q3 | K | V]
QTS = 512  # q tile size (free dim)
KTS = 128  # k tile size (partitions)


def build_program(
    s=S,
    d=D,
    phase_log=None,
    lag=1,
    wo_rot=False,
    norm_pool=False,
    rope_evac_dve=False,
    mask_split=True,
    vt_evac_dve=False,
    osum_act=False,
):
    import concourse.bass as bass
    import concourse.mybir as mybir
    import concourse.tile as tile
    from concourse import bacc

    def mark(label):
        if phase_log is not None:
            phase_log.append((label, len(nc.inst_map)))

    f32 = mybir.dt.float32
    bf16 = mybir.dt.bfloat16
    Exp = mybir.ActivationFunctionType.Exp
    Copy = mybir.ActivationFunctionType.Copy
    add_op = mybir.AluOpType.add
    mult_op = mybir.AluOpType.mult

    n_qt = s // QTS  # q tiles per batch (4)
    n_dkt = d // 128  # contraction tiles for projections (16)
    n_skt = s // KTS  # k tiles per batch (16)
    n_mo = (HQ * HD) // 128  # wo contraction tiles (2)

    nc = bacc.Bacc("TRN2", num_devices=N_CORES)
    xT_d = nc.declare_dram_parameter("xT", [B, d, s], bf16, isOutput=False)
    # weights pre-arranged host-side into SBUF layout [partition, kt, cols]
    # so the loads are single contiguous-per-partition DMA descriptors
    wkv_d = nc.declare_dram_parameter("wkv_r", [128, n_dkt * 128], bf16, isOutput=False)
    wq_d = nc.declare_dram_parameter("wq_r", [128, n_dkt * 256], bf16, isOutput=False)
    wo_d = nc.declare_dram_parameter("wo_r", [128, n_mo * d], bf16, isOutput=False)
    cos_d = nc.declare_dram_parameter("cosb", [128, s], bf16, isOutput=False)
    sin_d = nc.declare_dram_parameter("sinb", [128, s], bf16, isOutput=False)
    tri_d = nc.declare_dram_parameter("tri128", [128, 128], bf16, isOutput=False)
    part_d = nc.declare_dram_parameter("part", [B * s, d], bf16, isOutput=True)

    with tile.TileContext(nc) as tc:
        with (
            tc.tile_pool(name="const", bufs=1) as cpool,
            tc.tile_pool(name="big", bufs=1) as bpool,
            tc.tile_pool(name="work", bufs=3) as wpool,
            tc.tile_pool(name="estrip", bufs=8) as epool,
            tc.tile_pool(name="outp", bufs=4) as opool,
            tc.tile_pool(name="norm", bufs=3) as rpool,
            tc.tile_pool(name="pssc", bufs=2, space="PSUM") as pssc,
            tc.tile_pool(name="psops", bufs=2, space="PSUM") as psops,
            tc.tile_pool(name="psw", bufs=2, space="PSUM") as psw,
        ):
            # ---- constants / weights ----
            cos_sb = cpool.tile([128, s], bf16)
            sin_sb = cpool.tile([128, s], bf16)
            tri_sb = cpool.tile([128, 128], bf16)
            wkv_sb = cpool.tile([128, n_dkt, 128], bf16)
            wq_sb = cpool.tile([128, n_dkt, 256], bf16)
            wo_sb = cpool.tile([128, n_mo, d], bf16)

            # K/V weight columns first so the first projection can start as
            # soon as the first x chunk lands; Q columns + wo arrive behind it.
            nc.sync.dma_start(wkv_sb[:, :, :], wkv_d[:, :])

            tiles = {}

            def get_batch_tiles(b):
                if ("xT", b) not in tiles:
                    tiles[("xT", b)] = bpool.tile(
                        [128, n_dkt, s], bf16, tag="xT", name=f"xT{b}"
                    )
                    tiles[("QT", b)] = bpool.tile(
                        [128, n_mo, s], bf16, tag="QT", name=f"QT{b}"
                    )
                    tiles[("KT2", b)] = bpool.tile(
                        [128, s], bf16, tag="KT2", bufs=2, name=f"KT2{b}"
                    )
                    tiles[("VT", b)] = bpool.tile(
                        [128, s], bf16, tag="VT", bufs=2, name=f"VT{b}"
                    )
                    tiles[("V", b)] = bpool.tile(
                        [128, n_skt, 128], bf16, tag="V", bufs=2, name=f"V{b}"
                    )
                    tiles[("OT", b)] = bpool.tile(
                        [128, n_mo, s], bf16, tag="OT", bufs=2, name=f"OT{b}"
                    )
                return tiles

            def load_x_chunk(b, n, split=1):
                """DMA one token-tile chunk of x^T: all d-tiles, cols nsl.

                split>1 breaks it into several DMAs so latency-critical
                transfers (V transposes, outputs) don't queue behind one
                long transfer on the DMA engines.
                """
                xT_sb = get_batch_tiles(b)[("xT", b)]
                nsl = slice(n * QTS, (n + 1) * QTS)
                step = n_dkt // split
                for i in range(split):
                    ksl = slice(i * step, (i + 1) * step)
                    nc.sync.dma_start(
                        xT_sb[:, ksl, nsl],
                        xT_d[b, i * step * 128 : (i + 1) * step * 128, nsl].rearrange(
                            "(j p) c -> p j c", p=128
                        ),
                    )

            def rope_pair(dst, ps_src, rows, nsl, swaps):
                """RoPE on `rows` partitions of a psum tile into dst cols nsl."""
                r = slice(0, rows)
                q_raw = wpool.tile([128, QTS], bf16, tag="qraw")
                if rope_evac_dve:
                    nc.vector.tensor_copy(q_raw[r, :], ps_src[r, :])
                else:
                    nc.scalar.activation(q_raw[r, :], ps_src[r, :], Copy)
                t1 = wpool.tile([128, QTS], bf16, tag="t1")
                t2 = wpool.tile([128, QTS], bf16, tag="t2")
                nc.vector.tensor_tensor(t1[r, :], q_raw[r, :], cos_sb[r, nsl], mult_op)
                qsw = wpool.tile([128, QTS], bf16, tag="qsw")
                for r0, r1 in swaps:
                    nc.vector.tensor_copy(qsw[r0 : r0 + 32, :], q_raw[r1 : r1 + 32, :])
                nc.vector.tensor_tensor(t2[r, :], qsw[r, :], sin_sb[r, nsl], mult_op)
                nc.vector.tensor_tensor(dst, t1[r, :], t2[r, :], add_op)

            def kv_proj_block(b, n):
                """K/V projection for token tile n: 16 matmuls + evac."""
                mark(f"b{b}n{n}_kv")
                bt = get_batch_tiles(b)
                xT_sb = bt[("xT", b)]
                KT2_sb = bt[("KT2", b)]
                VT_sb = bt[("VT", b)]
                V_sb = bt[("V", b)]
                nsl = slice(n * QTS, (n + 1) * QTS)
                if n == 0:
                    # ones column / zero pad for PV lhsT
                    nc.gpsimd.memset(V_sb[:, :, 64:128], 0.0)
                    nc.gpsimd.memset(V_sb[:, :, 64:65], 1.0)
                ps = psw.tile([128, QTS], f32, tag="w")
                for kt in range(n_dkt):
                    nc.tensor.matmul(
                        ps[:],
                        wkv_sb[:, kt, :],
                        xT_sb[:, kt, nsl],
                        start=(kt == 0),
                        stop=(kt == n_dkt - 1),
                    )
                # rows 0:64 = K^T (rope), rows 64:128 = V^T (copy)
                rope_pair(KT2_sb[0:64, nsl], ps, 64, nsl, ((0, 32), (32, 0)))
                # duplicate K^T into partitions 64:128 (row-group packing)
                nc.vector.tensor_copy(KT2_sb[64:128, nsl], KT2_sb[0:64, nsl])
                # V^T: plain cast copy into partitions 64:128
                if vt_evac_dve:
                    nc.vector.tensor_copy(VT_sb[64:128, nsl], ps[64:128, :])
                else:
                    nc.scalar.activation(VT_sb[64:128, nsl], ps[64:128, :], Copy)
                # V^T -> V (token-major) via DMA transpose
                for kt in range(n * 4, n * 4 + 4):
                    nc.sync.dma_start_transpose(
                        V_sb[:, kt, 0:64],
                        VT_sb[64:128, kt * KTS : (kt + 1) * KTS],
                    )

            def q_proj_block(b, n, m):
                """Q projection for head pair m (heads 2m, 2m+1), token tile n."""
                bt = get_batch_tiles(b)
                xT_sb = bt[("xT", b)]
                QT_sb = bt[("QT", b)]
                nsl = slice(n * QTS, (n + 1) * QTS)
                ps = psw.tile([128, QTS], f32, tag="w")
                for kt in range(n_dkt):
                    nc.tensor.matmul(
                        ps[:],
                        wq_sb[:, kt, m * 128 : (m + 1) * 128],
                        xT_sb[:, kt, nsl],
                        start=(kt == 0),
                        stop=(kt == n_dkt - 1),
                    )
                rope_pair(
                    QT_sb[:, m, nsl], ps, 128, nsl, ((0, 32), (32, 0), (64, 96), (96, 64))
                )

            # ---- filler machinery: closures emitted between attention steps ----
            wo_queue = deque()
            pre_fillers = deque()  # emitted with priority over wo units

            def wo_unit(b, mt, nw, drain=False):
                """One wo output tile [128 tokens, 512 d-cols]."""
                OT_sb = tiles[("OT", b)]
                msl = slice(mt * 128, (mt + 1) * 128)
                nsl = slice(nw * QTS, (nw + 1) * QTS)
                osb = tiles.get(("osb", b, mt))
                if osb is None:
                    osb = opool.tile([128, d], bf16, tag="osb", name=f"osb{b}_{mt}")
                    tiles[("osb", b, mt)] = osb
                if drain:
                    # attention PSUM pools are idle during the final drain;
                    # borrow them so more units can be in flight
                    pool, tg = ((pssc, "sc"), (psops, "ops"), (psw, "w"))[nw % 3]
                    ps = pool.tile([128, QTS], f32, tag=tg)
                else:
                    ps = psw.tile([128, QTS], f32, tag="w")
                for kt in range(n_mo):
                    nc.tensor.matmul(
                        ps[:],
                        OT_sb[:, kt, msl],
                        wo_sb[:, kt, nsl],
                        start=(kt == 0),
                        stop=(kt == n_mo - 1),
                    )
                # during drain, alternate evacuation engines so units pipeline
                # instead of serializing behind one engine's queue (GPSIMD
                # cannot read PSUM, so only Act/DVE are eligible)
                if (drain or wo_rot) and nw % 2 == 1:
                    nc.scalar.activation(osb[:, nsl], ps[:], Copy)
                else:
                    nc.vector.tensor_copy(osb[:, nsl], ps[:])
                if nw == 1:
                    nc.sync.dma_start(
                        part_d[b * s + mt * 128 : b * s + (mt + 1) * 128, 0:1024],
                        osb[:, 0:1024],
                    )
                if nw == d // QTS - 1:
                    nc.sync.dma_start(
                        part_d[b * s + mt * 128 : b * s + (mt + 1) * 128, 1024:d],
                        osb[:, 1024:d],
                    )
                    del tiles[("osb", b, mt)]

            def pop_filler(k=1, drain=False):
                for _ in range(k):
                    if pre_fillers:
                        pre_fillers.popleft()()
                    elif wo_queue:
                        b_, mt_, nw_ = wo_queue.popleft()
                        wo_unit(b_, mt_, nw_, drain=drain)

            def queue_wo(b, qt):
                for mt in range(4 * qt, 4 * qt + 4):
                    for nw in range(d // QTS):
                        wo_queue.append((b, mt, nw))

            # ---- attention ----
            def attn_qtile(b, qt, evac_parity):
                mark(f"b{b}_attn{qt}")
                bt = get_batch_tiles(b)
                QT_sb = bt[("QT", b)]
                KT2_sb = bt[("KT2", b)]
                V_sb = bt[("V", b)]
                OT_sb = bt[("OT", b)]
                n_kt = (qt + 1) * (QTS // KTS)  # k tiles needed
                G = n_kt // 2  # strip groups of 2 k-tiles
                qsl = slice(qt * QTS, (qt + 1) * QTS)

                def emit_scores(h, g, sc, e):
                    hb = (h % 2) * 64
                    qh = QT_sb[hb : hb + 64, h // 2, :]
                    kt2 = KT2_sb[hb : hb + 64, :]
                    los = []
                    for j in (0, 1):
                        kt = 2 * g + j
                        o = kt * KTS - qt * QTS
                        lo = max(0, o)
                        los.append(lo)
                        nc.tensor.matmul(
                            sc[:, j, lo:QTS],
                            kt2[:, kt * KTS : (kt + 1) * KTS],
                            qh[:, qt * QTS + lo : (qt + 1) * QTS],
                            start=True,
                            stop=True,
                        )
                    # exp (clipped); diagonal windows get 0/1 lower-tri mask
                    if los[0] == 0 and los[1] == 0 and 2 * g + 1 < 4 * qt:
                        nc.scalar.activation(e[:, :, :], sc[:, :, :], Exp)
                    else:
                        for j in (0, 1):
                            nc.scalar.activation(
                                e[:, j, los[j] : QTS], sc[:, j, los[j] : QTS], Exp
                            )
                    for j in (0, 1):
                        kt = 2 * g + j
                        o = kt * KTS - qt * QTS
                        if o >= 0:
                            eng = nc.vector if (mask_split and j == 0) else nc.gpsimd
                            eng.tensor_tensor(
                                e[:, j, o : o + KTS],
                                e[:, j, o : o + KTS],
                                tri_sb[:],
                                mult_op,
                            )
                    return los

                def emit_pv(h, g, e, los, ops):
                    for j in (0, 1):
                        kt = 2 * g + j
                        lo = los[j]
                        nc.tensor.matmul(
                            ops[:, lo:QTS],
                            V_sb[:, kt, :],
                            e[:, j, lo:QTS],
                            start=(kt == 0),
                            stop=(kt == n_kt - 1),
                        )

                for pair in (0, 1):
                    heads = (2 * pair, 2 * pair + 1)
                    ops = {}
                    pend = {}  # (h, g) -> (e, los) awaiting PV
                    for h in heads:
                        ops[h] = psops.tile(
                            [128, QTS], f32, tag="ops", name=f"ops{h}"
                        )
                    for g in range(G + lag):
                        for h in heads:
                            if g < G:
                                sc = pssc.tile([128, 2, QTS], f32, tag="sc")
                                e = epool.tile([128, 2, QTS], bf16, tag="e")
                                los = emit_scores(h, g, sc, e)
                                pend[(h, g)] = (e, los)
                        for h in heads:
                            if g >= lag:
                                e, los = pend.pop((h, g - lag))
                                emit_pv(h, g - lag, e, los, ops[h])
                        pop_filler(1)
                    # normalize: evacuate O^T+sumexp to SBUF, recip, broadcast,
                    # scale into OT (broadcast + scale on Pool, off the DVE
                    # critical path)
                    for h in heads:
                        hb = (h % 2) * 64
                        osum = rpool.tile([72, QTS], f32, tag="osum")
                        if osum_act:
                            nc.scalar.activation(osum[0:65, :], ops[h][0:65, :], Copy)
                        else:
                            nc.vector.tensor_copy(osum[0:65, :], ops[h][0:65, :])
                        rt = rpool.tile([1, QTS], f32, tag="rt")
                        nc.vector.reciprocal(rt[:], osum[64:65, :])
                        bsb = rpool.tile([64, QTS], f32, tag="bsb")
                        nc.gpsimd.partition_broadcast(bsb[:], rt[:])
                        eng = nc.gpsimd if norm_pool else nc.vector
                        eng.tensor_tensor(
                            OT_sb[hb : hb + 64, h // 2, qsl],
                            osum[0:64, :],
                            bsb[:],
                            mult_op,
                        )
                    pop_filler(1)

            # ---------------- schedule ----------------
            mark("x0_load")
            # first chunk split in half so the first projection matmuls can
            # begin while the second half is still in flight; cos/sin for the
            # first token tile split off so RoPE isn't blocked behind the
            # full tables
            xT0 = get_batch_tiles(0)[("xT", 0)]
            nc.sync.dma_start(
                xT0[:, 0:8, 0:QTS],
                xT_d[0, 0:1024, 0:QTS].rearrange("(j p) c -> p j c", p=128),
            )
            nc.sync.dma_start(
                xT0[:, 8:16, 0:QTS],
                xT_d[0, 1024:2048, 0:QTS].rearrange("(j p) c -> p j c", p=128),
            )
            nc.sync.dma_start(cos_sb[:, 0:QTS], cos_d[:, 0:QTS])
            nc.sync.dma_start(sin_sb[:, 0:QTS], sin_d[:, 0:QTS])
            nc.sync.dma_start(wq_sb[:, :, :], wq_d[:, :])
            nc.sync.dma_start(tri_sb[:], tri_d[:])
            nc.sync.dma_start(cos_sb[:, QTS:s], cos_d[:, QTS:s])
            nc.sync.dma_start(sin_sb[:, QTS:s], sin_d[:, QTS:s])
            for n in range(1, n_qt):
                load_x_chunk(0, n)
            nc.sync.dma_start(wo_sb[:, :, :], wo_d[:, :])
            # projections run one token tile ahead of attention so the RoPE /
            # V-transpose chains finish during the previous attention block;
            # b1's first projections interleave into b0's last attention block
            for b in (0, 1):
                for n in range(n_qt):
                    if not (b == 1 and n == 0):
                        kv_proj_block(b, n)
                        q_proj_block(b, n, 0)
                        pop_filler(1)
                        q_proj_block(b, n, 1)
                    pop_filler(1)
                    if n > 0:
                        attn_qtile(b, n - 1, evac_parity=n % 2)
                        queue_wo(b, n - 1)
                    if b == 0:
                        # after attn so the next kv block's V transposes are
                        # not queued behind these long transfers
                        load_x_chunk(1, n, split=4)
                if b == 0:
                    pre_fillers.append(lambda: kv_proj_block(1, 0))
                    pre_fillers.append(lambda: q_proj_block(1, 0, 0))
                    pre_fillers.append(lambda: q_proj_block(1, 0, 1))
                attn_qtile(b, n_qt - 1, evac_parity=0)
                queue_wo(b, n_qt - 1)
                while pre_fillers:
                    pre_fillers.popleft()()
            mark("drain")
            while wo_queue:
                pop_filler(1, drain=True)
    mark("end")
    nc.compile()
    return nc


# ---------------- host-side sharding ----------------

_PERM = np.concatenate([np.arange(0, HD, 2), np.arange(1, HD, 2)])  # evens, odds


def make_core_inputs(x, freqs_cos, freqs_sin, wq, wk, wv, wo, s=S, d=D):
    """Build per-core input maps (list of dicts, one per core)."""
    xT = np.ascontiguousarray(np.transpose(x, (0, 2, 1))).astype(BF16)  # [B, D, S]

    cosT = np.ascontiguousarray(freqs_cos.T)  # [32, S]
    sinT = np.ascontiguousarray(freqs_sin.T)
    cosb = np.tile(np.concatenate([cosT, cosT], axis=0), (2, 1)).astype(BF16)  # [128,S]
    sinb = np.tile(np.concatenate([-sinT, sinT], axis=0), (2, 1)).astype(BF16)

    p = np.arange(128)[:, None]
    c = np.arange(128)[None, :]
    tri128 = np.where(c >= p, 1.0, 0.0).astype(BF16)

    scale = 1.0 / math.sqrt(HD)
    in_maps = []
    for cidx in range(N_CORES):
        wq_c = np.concatenate(
            [
                wq[:, (4 * cidx + h) * HD : (4 * cidx + h + 1) * HD][:, _PERM]
                for h in range(HQ)
            ],
            axis=1,
        ) * scale
        wk_c = wk[:, cidx * HD : (cidx + 1) * HD][:, _PERM]
        wv_c = wv[:, cidx * HD : (cidx + 1) * HD]
        wkv_c = np.concatenate([wk_c, wv_c], axis=1)  # [D, 128]
        wo_c = wo[4 * cidx * HD : (4 * cidx + HQ) * HD, :]  # [256, D]
        # rearrange into SBUF layout [partition, kt*cols] so each load is one
        # contiguous-per-partition DMA
        wkv_r = np.ascontiguousarray(
            wkv_c.reshape(16, 128, 128).transpose(1, 0, 2).reshape(128, -1)
        ).astype(BF16)
        wq_r = np.ascontiguousarray(
            wq_c.reshape(16, 128, 256).transpose(1, 0, 2).reshape(128, -1)
        ).astype(BF16)
        wo_r = np.ascontiguousarray(
            wo_c.reshape(2, 128, D).transpose(1, 0, 2).reshape(128, -1)
        ).astype(BF16)
        in_maps.append(
            {
                "xT": xT,
                "wkv_r": wkv_r,
                "wq_r": wq_r,
                "wo_r": wo_r,
                "cosb": cosb,
                "sinb": sinb,
                "tri128": tri128,
            }
        )
    return in_maps


_NC_CACHE = {}


def kernel(x, freqs_cos, freqs_sin, wq, wk, wv, wo):
    from concourse.bass_utils import run_bass_kernel_spmd

    x = np.asarray(x, np.float32)
    freqs_cos = np.asarray(freqs_cos, np.float32)
    freqs_sin = np.asarray(freqs_sin, np.float32)
    wq = np.asarray(wq, np.float32)
    wk = np.asarray(wk, np.float32)
    wv = np.asarray(wv, np.float32)
    wo = np.asarray(wo, np.float32)

    if "nc" not in _NC_CACHE:
        _NC_CACHE["nc"] = build_program()
    nc = _NC_CACHE["nc"]

    in_maps = make_core_inputs(x, freqs_cos, freqs_sin, wq, wk, wv, wo)
    res = run_bass_kernel_spmd(nc, in_maps, list(range(N_CORES)))
    acc = np.zeros((B * S, D), np.float32)
    for r in res.results:
        acc += np.asarray(r["part"], np.float32)
    return acc.reshape(B, S, D).astype(BF16)


# revision 45
# speedup vs baseline: 1.0167x; 1.0026x over previous
"""Trainium2 Bass kernel for nn_Attention_78151224918608.

Dense transformer attention block: QKV proj + RoPE + GQA causal attention
+ output proj. Sharding: tensor-parallel over heads across 8 cores
(core c: Q heads 4c..4c+3, KV head c). Each core computes a partial
output (its heads through wo rows); host sums the 8 bf16 partials in
fp32 and casts to bf16.

Layout strategy (per core, per batch):
  - All matmul operands bf16; accumulation fp32 in PSUM.
  - Projections computed transposed: QKV^T[384, S] = wqkv^T @ x^T so that
    Q^T/K^T (head-dim on partitions) feed the scores matmul directly.
  - RoPE: even/odd pair interleave is folded into wq/wk/wo columns on the
    host (perm = evens-then-odds), turning the pair swap into a 32-row
    block swap done with cross-partition copies on DVE.
  - Scores computed transposed per (b,h): S^T[k,q] = K^T.T @ Q^T, so the
    softmax denominator and P@V both contract over k = partitions:
    PV lhsT = [V | ones-col] gives O^T rows 0:64 and sumexp in row 64.
  - Causal: scores/exp/PV matmuls are column-clipped to the staircase;
    diagonal 128x128 windows get a 0/1 lower-tri multiply after exp.
  - Schedule: x is DMA-streamed n-major (token-tile chunks of all 16
    d-tiles); per token tile: KV proj -> Q proj -> attention for that
    q-tile, with output-projection (wo) work for the previous q-tile
    interleaved between attention pipeline steps to keep PE fed while
    the Activation engine runs exp. b1's x load and projections overlap
    b0's attention (KT2/VT/V/OT tiles double-buffered).
"""

import sys

sys.path.insert(0, "/opt/trn_rl_repo")

import math
from collections import deque
import numpy as np
import ml_dtypes

BF16 = ml_dtypes.bfloat16

# Problem constants (hardcoded per contract).
B = 2
S = 2048
D = 2048
N_HEADS = 32
N_KV_HEADS = 8
HD = 64
N_CORES = 8
HQ = N_HEADS // N_CORES  # 4 q heads per core
M_PROJ = HQ * HD + 2 * HD  # 384: [Q0 Q1 Q2 Q3 | K | V]
QTS = 512  # q tile size (free dim)
KTS = 128  # k tile size (partitions)


def build_program(
    s=S,
    d=D,
    phase_log=None,
    lag=1,
    wo_rot=False,
    norm_pool=False,
    rope_evac_dve=False,
    mask_split=True,
    vt_evac_dve=False,
    osum_act=False,
):
    import concourse.bass as bass
    import concourse.mybir as mybir
    import concourse.tile as tile
    from concourse import bacc

    def mark(label):
        if phase_log is not None:
            phase_log.append((label, len(nc.inst_map)))

    f32 = mybir.dt.float32
    bf16 = mybir.dt.bfloat16
    Exp = mybir.ActivationFunctionType.Exp
    Copy = mybir.ActivationFunctionType.Copy
    add_op = mybir.AluOpType.add
    mult_op = mybir.AluOpType.mult

    n_qt = s // QTS  # q tiles per batch (4)
    n_dkt = d // 128  # contraction tiles for projections (16)
    n_skt = s // KTS  # k tiles per batch (16)
    n_mo = (HQ * HD) // 128  # wo contraction tiles (2)

    nc = bacc.Bacc("TRN2", num_devices=N_CORES)
    xT_d = nc.declare_dram_parameter("xT", [B, d, s], bf16, isOutput=False)
    # weights pre-arranged host-side into SBUF layout [partition, kt, cols]
    # so the loads are single contiguous-per-partition DMA descriptors
    wkv_d = nc.declare_dram_parameter("wkv_r", [128, n_dkt * 128], bf16, isOutput=False)
    wq_d = nc.declare_dram_parameter("wq_r", [128, n_dkt * 256], bf16, isOutput=False)
    wo_d = nc.declare_dram_parameter("wo_r", [128, n_mo * d], bf16, isOutput=False)
    cos_d = nc.declare_dram_parameter("cosb", [128, s], bf16, isOutput=False)
    sin_d = nc.declare_dram_parameter("sinb", [128, s], bf16, isOutput=False)
    tri_d = nc.declare_dram_parameter("tri128", [128, 128], bf16, isOutput=False)
    part_d = nc.declare_dram_parameter("part", [B * s, d], bf16, isOutput=True)

    with tile.TileContext(nc) as tc:
        with (
            tc.tile_pool(name="const", bufs=1) as cpool,
            tc.tile_pool(name="big", bufs=1) as bpool,
            tc.tile_pool(name="work", bufs=3) as wpool,
            tc.tile_pool(name="estrip", bufs=8) as epool,
            tc.tile_pool(name="outp", bufs=4) as opool,
            tc.tile_pool(name="norm", bufs=3) as rpool,
            tc.tile_pool(name="pssc", bufs=2, space="PSUM") as pssc,
            tc.tile_pool(name="psops", bufs=2, space="PSUM") as psops,
            tc.tile_pool(name="psw", bufs=2, space="PSUM") as psw,
        ):
            # ---- constants / weights ----
            cos_sb = cpool.tile([128, s], bf16)
            sin_sb = cpool.tile([128, s], bf16)
            tri_sb = cpool.tile([128, 128], bf16)
            wkv_sb = cpool.tile([128, n_dkt, 128], bf16)
            wq_sb = cpool.tile([128, n_dkt, 256], bf16)
            wo_sb = cpool.tile([128, n_mo, d], bf16)

            # K/V weight columns first so the first projection can start as
            # soon as the first x chunk lands; Q columns + wo arrive behind it.
            nc.sync.dma_start(wkv_sb[:, :, :], wkv_d[:, :])

            tiles = {}

            def get_batch_tiles(b):
                if ("xT", b) not in tiles:
                    tiles[("xT", b)] = bpool.tile(
                        [128, n_dkt, s], bf16, tag="xT", name=f"xT{b}"
                    )
                    tiles[("QT", b)] = bpool.tile(
                        [128, n_mo, s], bf16, tag="QT", name=f"QT{b}"
                    )
                    tiles[("KT2", b)] = bpool.tile(
                        [128, s], bf16, tag="KT2", bufs=2, name=f"KT2{b}"
                    )
                    tiles[("VT", b)] = bpool.tile(
                        [128, s], bf16, tag="VT", bufs=2, name=f"VT{b}"
                    )
                    tiles[("V", b)] = bpool.tile(
                        [128, n_skt, 128], bf16, tag="V", bufs=2, name=f"V{b}"
                    )
                    tiles[("OT", b)] = bpool.tile(
                        [128, n_mo, s], bf16, tag="OT", bufs=2, name=f"OT{b}"
                    )
                return tiles

            def load_x_chunk(b, n, split=1):
                """DMA one token-tile chunk of x^T: all d-tiles, cols nsl.

                split>1 breaks it into several DMAs so latency-critical
                transfers (V transposes, outputs) don't queue behind one
                long transfer on the DMA engines.
                """
                xT_sb = get_batch_tiles(b)[("xT", b)]
                nsl = slice(n * QTS, (n + 1) * QTS)
                step = n_dkt // split
                for i in range(split):
                    ksl = slice(i * step, (i + 1) * step)
                    nc.sync.dma_start(
                        xT_sb[:, ksl, nsl],
                        xT_d[b, i * step * 128 : (i + 1) * step * 128, nsl].rearrange(
                            "(j p) c -> p j c", p=128
                        ),
                    )

            def rope_pair(dst, ps_src, rows, nsl, swaps):
                """RoPE on `rows` partitions of a psum tile into dst cols nsl."""
                r = slice(0, rows)
                q_raw = wpool.tile([128, QTS], bf16, tag="qraw")
                if rope_evac_dve:
                    nc.vector.tensor_copy(q_raw[r, :], ps_src[r, :])
                else:
                    nc.scalar.activation(q_raw[r, :], ps_src[r, :], Copy)
                t1 = wpool.tile([128, QTS], bf16, tag="t1")
                t2 = wpool.tile([128, QTS], bf16, tag="t2")
                nc.vector.tensor_tensor(t1[r, :], q_raw[r, :], cos_sb[r, nsl], mult_op)
                qsw = wpool.tile([128, QTS], bf16, tag="qsw")
                for r0, r1 in swaps:
                    nc.vector.tensor_copy(qsw[r0 : r0 + 32, :], q_raw[r1 : r1 + 32, :])
                nc.vector.tensor_tensor(t2[r, :], qsw[r, :], sin_sb[r, nsl], mult_op)
                nc.vector.tensor_tensor(dst, t1[r, :], t2[r, :], add_op)

            def kv_proj_block(b, n):
                """K/V projection for token tile n: 16 matmuls + evac."""
                mark(f"b{b}n{n}_kv")
                bt = get_batch_tiles(b)
                xT_sb = bt[("xT", b)]
                KT2_sb = bt[("KT2", b)]
                VT_sb = bt[("VT", b)]
                V_sb = bt[("V", b)]
                nsl = slice(n * QTS, (n + 1) * QTS)
                ps = psw.tile([128, QTS], f32, tag="w")
                for kt in range(n_dkt):
                    nc.tensor.matmul(
                        ps[:],
                        wkv_sb[:, kt, :],
                        xT_sb[:, kt, nsl],
                        start=(kt == 0),
                        stop=(kt == n_dkt - 1),
                    )
                # rows 0:64 = K^T (rope), rows 64:128 = V^T (copy)
                rope_pair(KT2_sb[0:64, nsl], ps, 64, nsl, ((0, 32), (32, 0)))
                # duplicate K^T into partitions 64:128 (row-group packing)
                nc.vector.tensor_copy(KT2_sb[64:128, nsl], KT2_sb[0:64, nsl])
                # V^T: plain cast copy into partitions 64:128
                if vt_evac_dve:
                    nc.vector.tensor_copy(VT_sb[64:128, nsl], ps[64:128, :])
                else:
                    nc.scalar.activation(VT_sb[64:128, nsl], ps[64:128, :], Copy)
                # V^T -> V (token-major) via DMA transpose
                for kt in range(n * 4, n * 4 + 4):
                    nc.sync.dma_start_transpose(
                        V_sb[:, kt, 0:64],
                        VT_sb[64:128, kt * KTS : (kt + 1) * KTS],
                    )

            def q_proj_block(b, n, m):
                """Q projection for head pair m (heads 2m, 2m+1), token tile n."""
                bt = get_batch_tiles(b)
                xT_sb = bt[("xT", b)]
                QT_sb = bt[("QT", b)]
                nsl = slice(n * QTS, (n + 1) * QTS)
                ps = psw.tile([128, QTS], f32, tag="w")
                for kt in range(n_dkt):
                    nc.tensor.matmul(
                        ps[:],
                        wq_sb[:, kt, m * 128 : (m + 1) * 128],
                        xT_sb[:, kt, nsl],
                        start=(kt == 0),
                        stop=(kt == n_dkt - 1),
                    )
                rope_pair(
                    QT_sb[:, m, nsl], ps, 128, nsl, ((0, 32), (32, 0), (64, 96), (96, 64))
                )

            # ---- filler machinery: closures emitted between attention steps ----
            wo_queue = deque()
            pre_fillers = deque()  # emitted with priority over wo units

            def wo_unit(b, mt, nw, drain=False):
                """One wo output tile [128 tokens, 512 d-cols]."""
                OT_sb = tiles[("OT", b)]
                msl = slice(mt * 128, (mt + 1) * 128)
                nsl = slice(nw * QTS, (nw + 1) * QTS)
                osb = tiles.get(("osb", b, mt))
                if osb is None:
                    osb = opool.tile([128, d], bf16, tag="osb", name=f"osb{b}_{mt}")
                    tiles[("osb", b, mt)] = osb
                if drain:
                    # attention PSUM pools are idle during the final drain;
                    # borrow them so more units can be in flight
                    pool, tg = ((pssc, "sc"), (psops, "ops"), (psw, "w"))[nw % 3]
                    ps = pool.tile([128, QTS], f32, tag=tg)
                else:
                    ps = psw.tile([128, QTS], f32, tag="w")
                for kt in range(n_mo):
                    nc.tensor.matmul(
                        ps[:],
                        OT_sb[:, kt, msl],
                        wo_sb[:, kt, nsl],
                        start=(kt == 0),
                        stop=(kt == n_mo - 1),
                    )
                # during drain, alternate evacuation engines so units pipeline
                # instead of serializing behind one engine's queue (GPSIMD
                # cannot read PSUM, so only Act/DVE are eligible)
                if (drain or wo_rot) and nw % 2 == 1:
                    nc.scalar.activation(osb[:, nsl], ps[:], Copy)
                else:
                    nc.vector.tensor_copy(osb[:, nsl], ps[:])
                if nw == 1:
                    nc.sync.dma_start(
                        part_d[b * s + mt * 128 : b * s + (mt + 1) * 128, 0:1024],
                        osb[:, 0:1024],
                    )
                if nw == d // QTS - 1:
                    nc.sync.dma_start(
                        part_d[b * s + mt * 128 : b * s + (mt + 1) * 128, 1024:d],
                        osb[:, 1024:d],
                    )
                    del tiles[("osb", b, mt)]

            def pop_filler(k=1, drain=False):
                for _ in range(k):
                    if pre_fillers:
                        pre_fillers.popleft()()
                    elif wo_queue:
                        b_, mt_, nw_ = wo_queue.popleft()
                        wo_unit(b_, mt_, nw_, drain=drain)

            def queue_wo(b, qt):
                for mt in range(4 * qt, 4 * qt + 4):
                    for nw in range(d // QTS):
                        wo_queue.append((b, mt, nw))

            # ---- attention ----
            def attn_qtile(b, qt, evac_parity):
                mark(f"b{b}_attn{qt}")
                bt = get_batch_tiles(b)
                QT_sb = bt[("QT", b)]
                KT2_sb = bt[("KT2", b)]
                V_sb = bt[("V", b)]
                OT_sb = bt[("OT", b)]
                n_kt = (qt + 1) * (QTS // KTS)  # k tiles needed
                G = n_kt // 2  # strip groups of 2 k-tiles
                qsl = slice(qt * QTS, (qt + 1) * QTS)

                def emit_scores(h, g, sc, e):
                    hb = (h % 2) * 64
                    qh = QT_sb[hb : hb + 64, h // 2, :]
                    kt2 = KT2_sb[hb : hb + 64, :]
                    los = []
                    for j in (0, 1):
                        kt = 2 * g + j
                        o = kt * KTS - qt * QTS
                        lo = max(0, o)
                        los.append(lo)
                        nc.tensor.matmul(
                            sc[:, j, lo:QTS],
                            kt2[:, kt * KTS : (kt + 1) * KTS],
                            qh[:, qt * QTS + lo : (qt + 1) * QTS],
                            start=True,
                            stop=True,
                        )
                    # exp (clipped); diagonal windows get 0/1 lower-tri mask
                    if los[0] == 0 and los[1] == 0 and 2 * g + 1 < 4 * qt:
                        nc.scalar.activation(e[:, :, :], sc[:, :, :], Exp)
                    else:
                        for j in (0, 1):
                            nc.scalar.activation(
                                e[:, j, los[j] : QTS], sc[:, j, los[j] : QTS], Exp
                            )
                    for j in (0, 1):
                        kt = 2 * g + j
                        o = kt * KTS - qt * QTS
                        if o >= 0:
                            eng = nc.vector if (mask_split and j == 0) else nc.gpsimd
                            eng.tensor_tensor(
                                e[:, j, o : o + KTS],
                                e[:, j, o : o + KTS],
                                tri_sb[:],
                                mult_op,
                            )
                    return los

                def emit_pv(h, g, e, los, ops):
                    for j in (0, 1):
                        kt = 2 * g + j
                        lo = los[j]
                        nc.tensor.matmul(
                            ops[:, lo:QTS],
                            V_sb[:, kt, :],
                            e[:, j, lo:QTS],
                            start=(kt == 0),
                            stop=(kt == n_kt - 1),
                        )

                for pair in (0, 1):
                    heads = (2 * pair, 2 * pair + 1)
                    ops = {}
                    pend = {}  # (h, g) -> (e, los) awaiting PV
                    for h in heads:
                        ops[h] = psops.tile(
                            [128, QTS], f32, tag="ops", name=f"ops{h}"
                        )
                    for g in range(G + lag):
                        for h in heads:
                            if g < G:
                                sc = pssc.tile([128, 2, QTS], f32, tag="sc")
                                e = epool.tile([128, 2, QTS], bf16, tag="e")
                                los = emit_scores(h, g, sc, e)
                                pend[(h, g)] = (e, los)
                        for h in heads:
                            if g >= lag:
                                e, los = pend.pop((h, g - lag))
                                emit_pv(h, g - lag, e, los, ops[h])
                        pop_filler(1)
                    # normalize: evacuate O^T+sumexp to SBUF, recip, broadcast,
                    # scale into OT (broadcast + scale on Pool, off the DVE
                    # critical path)
                    for h in heads:
                        hb = (h % 2) * 64
                        osum = rpool.tile([72, QTS], f32, tag="osum")
                        rt = rpool.tile([1, QTS], f32, tag="rt")
                        # recip reads the sumexp row straight from PSUM so it
                        # doesn't serialize behind the O^T evacuation
                        nc.vector.reciprocal(rt[:], ops[h][64:65, :])
                        if osum_act:
                            nc.scalar.activation(osum[0:64, :], ops[h][0:64, :], Copy)
                        else:
                            nc.vector.tensor_copy(osum[0:64, :], ops[h][0:64, :])
                        bsb = rpool.tile([64, QTS], f32, tag="bsb")
                        nc.gpsimd.partition_broadcast(bsb[:], rt[:])
                        eng = nc.gpsimd if norm_pool else nc.vector
                        eng.tensor_tensor(
                            OT_sb[hb : hb + 64, h // 2, qsl],
                            osum[0:64, :],
                            bsb[:],
                            mult_op,
                        )
                    pop_filler(1)

            # ---------------- schedule ----------------
            mark("x0_load")
            # ones column / zero pad for the PV lhsT of both batches, issued
            # first so nothing downstream waits on the Pool counter
            for b in (0, 1):
                V_b = get_batch_tiles(b)[("V", b)]
                nc.gpsimd.memset(V_b[:, :, 64:128], 0.0)
                nc.gpsimd.memset(V_b[:, :, 64:65], 1.0)
            # first chunk split in half so the first projection matmuls can
            # begin while the second half is still in flight; cos/sin for the
            # first token tile split off so RoPE isn't blocked behind the
            # full tables
            xT0 = get_batch_tiles(0)[("xT", 0)]
            nc.sync.dma_start(
                xT0[:, 0:8, 0:QTS],
                xT_d[0, 0:1024, 0:QTS].rearrange("(j p) c -> p j c", p=128),
            )
            nc.sync.dma_start(
                xT0[:, 8:16, 0:QTS],
                xT_d[0, 1024:2048, 0:QTS].rearrange("(j p) c -> p j c", p=128),
            )
            nc.sync.dma_start(cos_sb[:, 0:QTS], cos_d[:, 0:QTS])
            nc.sync.dma_start(sin_sb[:, 0:QTS], sin_d[:, 0:QTS])
            nc.sync.dma_start(wq_sb[:, :, :], wq_d[:, :])
            nc.sync.dma_start(tri_sb[:], tri_d[:])
            nc.sync.dma_start(cos_sb[:, QTS:s], cos_d[:, QTS:s])
            nc.sync.dma_start(sin_sb[:, QTS:s], sin_d[:, QTS:s])
            for n in range(1, n_qt):
                load_x_chunk(0, n)
            nc.sync.dma_start(wo_sb[:, :, :], wo_d[:, :])
            # projections run one token tile ahead of attention so the RoPE /
            # V-transpose chains finish during the previous attention block;
            # b1's first projections interleave into b0's last attention block
            for b in (0, 1):
                for n in range(n_qt):
                    if not (b == 1 and n == 0):
                        kv_proj_block(b, n)
                        q_proj_block(b, n, 0)
                        pop_filler(1)
                        q_proj_block(b, n, 1)
                    pop_filler(1)
                    if n > 0:
                        attn_qtile(b, n - 1, evac_parity=n % 2)
                        queue_wo(b, n - 1)
                    if b == 0:
                        # after attn so the next kv block's V transposes are
                        # not queued behind these long transfers
                        load_x_chunk(1, n, split=4)
                if b == 0:
                    pre_fillers.append(lambda: kv_proj_block(1, 0))
                    pre_fillers.append(lambda: q_proj_block(1, 0, 0))
                    pre_fillers.append(lambda: q_proj_block(1, 0, 1))
                attn_qtile(b, n_qt - 1, evac_parity=0)
                queue_wo(b, n_qt - 1)
                while pre_fillers:
                    pre_fillers.popleft()()
            mark("drain")
            while wo_queue:
                pop_filler(1, drain=True)
    mark("end")
    nc.compile()
    return nc


# ---------------- host-side sharding ----------------

_PERM = np.concatenate([np.arange(0, HD, 2), np.arange(1, HD, 2)])  # evens, odds


def make_core_inputs(x, freqs_cos, freqs_sin, wq, wk, wv, wo, s=S, d=D):
    """Build per-core input maps (list of dicts, one per core)."""
    xT = np.ascontiguousarray(np.transpose(x, (0, 2, 1))).astype(BF16)  # [B, D, S]

    cosT = np.ascontiguousarray(freqs_cos.T)  # [32, S]
    sinT = np.ascontiguousarray(freqs_sin.T)
    cosb = np.tile(np.concatenate([cosT, cosT], axis=0), (2, 1)).astype(BF16)  # [128,S]
    sinb = np.tile(np.concatenate([-sinT, sinT], axis=0), (2, 1)).astype(BF16)

    p = np.arange(128)[:, None]
    c = np.arange(128)[None, :]
    tri128 = np.where(c >= p, 1.0, 0.0).astype(BF16)

    scale = 1.0 / math.sqrt(HD)
    in_maps = []
    for cidx in range(N_CORES):
        wq_c = np.concatenate(
            [
                wq[:, (4 * cidx + h) * HD : (4 * cidx + h + 1) * HD][:, _PERM]
                for h in range(HQ)
            ],
            axis=1,
        ) * scale
        wk_c = wk[:, cidx * HD : (cidx + 1) * HD][:, _PERM]
        wv_c = wv[:, cidx * HD : (cidx + 1) * HD]
        wkv_c = np.concatenate([wk_c, wv_c], axis=1)  # [D, 128]
        wo_c = wo[4 * cidx * HD : (4 * cidx + HQ) * HD, :]  # [256, D]
        # rearrange into SBUF layout [partition, kt*cols] so each load is one
        # contiguous-per-partition DMA
        wkv_r = np.ascontiguousarray(
            wkv_c.reshape(16, 128, 128).transpose(1, 0, 2).reshape(128, -1)
        ).astype(BF16)
        wq_r = np.ascontiguousarray(
            wq_c.reshape(16, 128, 256).transpose(1, 0, 2).reshape(128, -1)
        ).astype(BF16)
        wo_r = np.ascontiguousarray(
            wo_c.reshape(2, 128, D).transpose(1, 0, 2).reshape(128, -1)
        ).astype(BF16)
        in_maps.append(
            {
                "xT": xT,
                "wkv_r": wkv_r,
                "wq_r": wq_r,
                "wo_r": wo_r,
                "cosb": cosb,
                "sinb": sinb,
                "tri128": tri128,
            }
        )
    return in_maps


_NC_CACHE = {}


def kernel(x, freqs_cos, freqs_sin, wq, wk, wv, wo):
    from concourse.bass_utils import run_bass_kernel_spmd

    x = np.asarray(x, np.float32)
    freqs_cos = np.asarray(freqs_cos, np.float32)
    freqs_sin = np.asarray(freqs_sin, np.float32)
    wq = np.asarray(wq, np.float32)
    wk = np.asarray(wk, np.float32)
    wv = np.asarray(wv, np.float32)
    wo = np.asarray(wo, np.float32)

    if "nc" not in _NC_CACHE:
        _NC_CACHE["nc"] = build_program()
    nc = _NC_CACHE["nc"]

    in_maps = make_core_inputs(x, freqs_cos, freqs_sin, wq, wk, wv, wo)
    res = run_bass_kernel_spmd(nc, in_maps, list(range(N_CORES)))
    acc = np.zeros((B * S, D), np.float32)
    for r in res.results:
        acc += np.asarray(r["part"], np.float32)
    return acc.reshape(B, S, D).astype(BF16)


# revision 52
# speedup vs baseline: 1.0206x; 1.0038x over previous
"""Trainium2 Bass kernel for nn_Attention_78151224918608.

Dense transformer attention block: QKV proj + RoPE + GQA causal attention
+ output proj. Sharding: tensor-parallel over heads across 8 cores
(core c: Q heads 4c..4c+3, KV head c). Each core computes a partial
output (its heads through wo rows); host sums the 8 bf16 partials in
fp32 and casts to bf16.

Layout strategy (per core, per batch):
  - All matmul operands bf16; accumulation fp32 in PSUM.
  - Projections computed transposed: QKV^T[384, S] = wqkv^T @ x^T so that
    Q^T/K^T (head-dim on partitions) feed the scores matmul directly.
  - RoPE: even/odd pair interleave is folded into wq/wk/wo columns on the
    host (perm = evens-then-odds), turning the pair swap into a 32-row
    block swap done with cross-partition copies on DVE.
  - Scores computed transposed per (b,h): S^T[k,q] = K^T.T @ Q^T, so the
    softmax denominator and P@V both contract over k = partitions:
    PV lhsT = [V | ones-col] gives O^T rows 0:64 and sumexp in row 64.
  - Causal: scores/exp/PV matmuls are column-clipped to the staircase;
    diagonal 128x128 windows get a 0/1 lower-tri multiply after exp.
  - Schedule: x is DMA-streamed n-major (token-tile chunks of all 16
    d-tiles); per token tile: KV proj -> Q proj -> attention for that
    q-tile, with output-projection (wo) work for the previous q-tile
    interleaved between attention pipeline steps to keep PE fed while
    the Activation engine runs exp. b1's x load and projections overlap
    b0's attention (KT2/VT/V/OT tiles double-buffered).
"""

import sys

sys.path.insert(0, "/opt/trn_rl_repo")

import math
from collections import deque
import numpy as np
import ml_dtypes

BF16 = ml_dtypes.bfloat16

# Problem constants (hardcoded per contract).
B = 2
S = 2048
D = 2048
N_HEADS = 32
N_KV_HEADS = 8
HD = 64
N_CORES = 8
HQ = N_HEADS // N_CORES  # 4 q heads per core
M_PROJ = HQ * HD + 2 * HD  # 384: [Q0 Q1 Q2 Q3 | K | V]
QTS = 512  # q tile size (free dim)
KTS = 128  # k tile size (partitions)


def build_program(
    s=S,
    d=D,
    phase_log=None,
    lag=1,
    wo_rot=False,
    norm_pool=False,
    rope_evac_dve=False,
    mask_split=2,
    vt_evac_dve=False,
    osum_act=False,
):
    import concourse.bass as bass
    import concourse.mybir as mybir
    import concourse.tile as tile
    from concourse import bacc

    def mark(label):
        if phase_log is not None:
            phase_log.append((label, len(nc.inst_map)))

    f32 = mybir.dt.float32
    bf16 = mybir.dt.bfloat16
    Exp = mybir.ActivationFunctionType.Exp
    Copy = mybir.ActivationFunctionType.Copy
    add_op = mybir.AluOpType.add
    mult_op = mybir.AluOpType.mult

    n_qt = s // QTS  # q tiles per batch (4)
    n_dkt = d // 128  # contraction tiles for projections (16)
    n_skt = s // KTS  # k tiles per batch (16)
    n_mo = (HQ * HD) // 128  # wo contraction tiles (2)

    nc = bacc.Bacc("TRN2", num_devices=N_CORES)
    xT_d = nc.declare_dram_parameter("xT", [B, d, s], bf16, isOutput=False)
    # weights pre-arranged host-side into SBUF layout [partition, kt, cols]
    # so the loads are single contiguous-per-partition DMA descriptors
    wkv_d = nc.declare_dram_parameter("wkv_r", [128, n_dkt * 128], bf16, isOutput=False)
    wq_d = nc.declare_dram_parameter("wq_r", [128, n_dkt * 256], bf16, isOutput=False)
    wo_d = nc.declare_dram_parameter("wo_r", [128, n_mo * d], bf16, isOutput=False)
    cos_d = nc.declare_dram_parameter("cosb", [128, s], bf16, isOutput=False)
    sin_d = nc.declare_dram_parameter("sinb", [128, s], bf16, isOutput=False)
    tri_d = nc.declare_dram_parameter("tri128", [128, 128], bf16, isOutput=False)
    part_d = nc.declare_dram_parameter("part", [B * s, d], bf16, isOutput=True)

    with tile.TileContext(nc) as tc:
        with (
            tc.tile_pool(name="const", bufs=1) as cpool,
            tc.tile_pool(name="big", bufs=1) as bpool,
            tc.tile_pool(name="work", bufs=3) as wpool,
            tc.tile_pool(name="estrip", bufs=8) as epool,
            tc.tile_pool(name="outp", bufs=4) as opool,
            tc.tile_pool(name="norm", bufs=3) as rpool,
            tc.tile_pool(name="pssc", bufs=2, space="PSUM") as pssc,
            tc.tile_pool(name="psops", bufs=2, space="PSUM") as psops,
            tc.tile_pool(name="psw", bufs=2, space="PSUM") as psw,
        ):
            # ---- constants / weights ----
            cos_sb = cpool.tile([128, s], bf16)
            sin_sb = cpool.tile([128, s], bf16)
            tri_sb = cpool.tile([128, 128], bf16)
            wkv_sb = cpool.tile([128, n_dkt, 128], bf16)
            wq_sb = cpool.tile([128, n_dkt, 256], bf16)
            wo_sb = cpool.tile([128, n_mo, d], bf16)

            # K/V weight columns first so the first projection can start as
            # soon as the first x chunk lands; Q columns + wo arrive behind it.
            nc.sync.dma_start(wkv_sb[:, :, :], wkv_d[:, :])

            tiles = {}

            def get_batch_tiles(b):
                if ("xT", b) not in tiles:
                    tiles[("xT", b)] = bpool.tile(
                        [128, n_dkt, s], bf16, tag="xT", name=f"xT{b}"
                    )
                    tiles[("QT", b)] = bpool.tile(
                        [128, n_mo, s], bf16, tag="QT", name=f"QT{b}"
                    )
                    tiles[("KT2", b)] = bpool.tile(
                        [128, s], bf16, tag="KT2", bufs=2, name=f"KT2{b}"
                    )
                    tiles[("VT", b)] = bpool.tile(
                        [128, s], bf16, tag="VT", bufs=2, name=f"VT{b}"
                    )
                    tiles[("V", b)] = bpool.tile(
                        [128, n_skt, 128], bf16, tag="V", bufs=2, name=f"V{b}"
                    )
                    tiles[("OT", b)] = bpool.tile(
                        [128, n_mo, s], bf16, tag="OT", bufs=2, name=f"OT{b}"
                    )
                return tiles

            def load_x_chunk(b, n, split=1):
                """DMA one token-tile chunk of x^T: all d-tiles, cols nsl.

                split>1 breaks it into several DMAs so latency-critical
                transfers (V transposes, outputs) don't queue behind one
                long transfer on the DMA engines.
                """
                xT_sb = get_batch_tiles(b)[("xT", b)]
                nsl = slice(n * QTS, (n + 1) * QTS)
                step = n_dkt // split
                for i in range(split):
                    ksl = slice(i * step, (i + 1) * step)
                    nc.sync.dma_start(
                        xT_sb[:, ksl, nsl],
                        xT_d[b, i * step * 128 : (i + 1) * step * 128, nsl].rearrange(
                            "(j p) c -> p j c", p=128
                        ),
                    )

            def rope_pair(dst, ps_src, rows, nsl, swaps, bb=0):
                """RoPE on `rows` partitions of a psum tile into dst cols nsl."""
                r = slice(0, rows)
                q_raw = wpool.tile([128, QTS], bf16, tag="qraw")
                if rope_evac_dve is True or (rope_evac_dve == "b1" and bb == 1):
                    nc.vector.tensor_copy(q_raw[r, :], ps_src[r, :])
                else:
                    nc.scalar.activation(q_raw[r, :], ps_src[r, :], Copy)
                t1 = wpool.tile([128, QTS], bf16, tag="t1")
                t2 = wpool.tile([128, QTS], bf16, tag="t2")
                nc.vector.tensor_tensor(t1[r, :], q_raw[r, :], cos_sb[r, nsl], mult_op)
                qsw = wpool.tile([128, QTS], bf16, tag="qsw")
                for r0, r1 in swaps:
                    nc.vector.tensor_copy(qsw[r0 : r0 + 32, :], q_raw[r1 : r1 + 32, :])
                nc.vector.tensor_tensor(t2[r, :], qsw[r, :], sin_sb[r, nsl], mult_op)
                nc.vector.tensor_tensor(dst, t1[r, :], t2[r, :], add_op)

            def kv_proj_block(b, n):
                """K/V projection for token tile n: 16 matmuls + evac."""
                mark(f"b{b}n{n}_kv")
                bt = get_batch_tiles(b)
                xT_sb = bt[("xT", b)]
                KT2_sb = bt[("KT2", b)]
                VT_sb = bt[("VT", b)]
                V_sb = bt[("V", b)]
                nsl = slice(n * QTS, (n + 1) * QTS)
                ps = psw.tile([128, QTS], f32, tag="w")
                for kt in range(n_dkt):
                    nc.tensor.matmul(
                        ps[:],
                        wkv_sb[:, kt, :],
                        xT_sb[:, kt, nsl],
                        start=(kt == 0),
                        stop=(kt == n_dkt - 1),
                    )
                # rows 0:64 = K^T (rope), rows 64:128 = V^T (copy)
                rope_pair(KT2_sb[0:64, nsl], ps, 64, nsl, ((0, 32), (32, 0)), bb=b)
                # duplicate K^T into partitions 64:128 (row-group packing)
                nc.vector.tensor_copy(KT2_sb[64:128, nsl], KT2_sb[0:64, nsl])
                # V^T: plain cast copy into partitions 64:128
                if vt_evac_dve:
                    nc.vector.tensor_copy(VT_sb[64:128, nsl], ps[64:128, :])
                else:
                    nc.scalar.activation(VT_sb[64:128, nsl], ps[64:128, :], Copy)
                # V^T -> V (token-major) via DMA transpose
                for kt in range(n * 4, n * 4 + 4):
                    nc.sync.dma_start_transpose(
                        V_sb[:, kt, 0:64],
                        VT_sb[64:128, kt * KTS : (kt + 1) * KTS],
                    )

            def q_proj_block(b, n, m):
                """Q projection for head pair m (heads 2m, 2m+1), token tile n."""
                bt = get_batch_tiles(b)
                xT_sb = bt[("xT", b)]
                QT_sb = bt[("QT", b)]
                nsl = slice(n * QTS, (n + 1) * QTS)
                ps = psw.tile([128, QTS], f32, tag="w")
                for kt in range(n_dkt):
                    nc.tensor.matmul(
                        ps[:],
                        wq_sb[:, kt, m * 128 : (m + 1) * 128],
                        xT_sb[:, kt, nsl],
                        start=(kt == 0),
                        stop=(kt == n_dkt - 1),
                    )
                rope_pair(
                    QT_sb[:, m, nsl],
                    ps,
                    128,
                    nsl,
                    ((0, 32), (32, 0), (64, 96), (96, 64)),
                    bb=b,
                )

            # ---- filler machinery: closures emitted between attention steps ----
            wo_queue = deque()
            pre_fillers = deque()  # emitted with priority over wo units

            def wo_unit(b, mt, nw, drain=False):
                """One wo output tile [128 tokens, 512 d-cols]."""
                OT_sb = tiles[("OT", b)]
                msl = slice(mt * 128, (mt + 1) * 128)
                nsl = slice(nw * QTS, (nw + 1) * QTS)
                osb = tiles.get(("osb", b, mt))
                if osb is None:
                    osb = opool.tile([128, d], bf16, tag="osb", name=f"osb{b}_{mt}")
                    tiles[("osb", b, mt)] = osb
                if drain:
                    # attention PSUM pools are idle during the final drain;
                    # borrow them so more units can be in flight
                    pool, tg = ((pssc, "sc"), (psops, "ops"), (psw, "w"))[nw % 3]
                    ps = pool.tile([128, QTS], f32, tag=tg)
                else:
                    ps = psw.tile([128, QTS], f32, tag="w")
                for kt in range(n_mo):
                    nc.tensor.matmul(
                        ps[:],
                        OT_sb[:, kt, msl],
                        wo_sb[:, kt, nsl],
                        start=(kt == 0),
                        stop=(kt == n_mo - 1),
                    )
                # during drain, alternate evacuation engines so units pipeline
                # instead of serializing behind one engine's queue (GPSIMD
                # cannot read PSUM, so only Act/DVE are eligible)
                if (drain or wo_rot) and nw % 2 == 1:
                    nc.scalar.activation(osb[:, nsl], ps[:], Copy)
                else:
                    nc.vector.tensor_copy(osb[:, nsl], ps[:])
                if nw == 1:
                    nc.sync.dma_start(
                        part_d[b * s + mt * 128 : b * s + (mt + 1) * 128, 0:1024],
                        osb[:, 0:1024],
                    )
                if nw == d // QTS - 1:
                    nc.sync.dma_start(
                        part_d[b * s + mt * 128 : b * s + (mt + 1) * 128, 1024:d],
                        osb[:, 1024:d],
                    )
                    del tiles[("osb", b, mt)]

            def pop_filler(k=1, drain=False):
                for _ in range(k):
                    if pre_fillers:
                        pre_fillers.popleft()()
                    elif wo_queue:
                        b_, mt_, nw_ = wo_queue.popleft()
                        wo_unit(b_, mt_, nw_, drain=drain)

            def queue_wo(b, qt):
                for mt in range(4 * qt, 4 * qt + 4):
                    for nw in range(d // QTS):
                        wo_queue.append((b, mt, nw))

            # ---- attention ----
            def attn_qtile(b, qt, evac_parity):
                mark(f"b{b}_attn{qt}")
                bt = get_batch_tiles(b)
                QT_sb = bt[("QT", b)]
                KT2_sb = bt[("KT2", b)]
                V_sb = bt[("V", b)]
                OT_sb = bt[("OT", b)]
                n_kt = (qt + 1) * (QTS // KTS)  # k tiles needed
                G = n_kt // 2  # strip groups of 2 k-tiles
                qsl = slice(qt * QTS, (qt + 1) * QTS)

                def emit_scores(h, g, sc, e):
                    hb = (h % 2) * 64
                    qh = QT_sb[hb : hb + 64, h // 2, :]
                    kt2 = KT2_sb[hb : hb + 64, :]
                    los = []
                    for j in (0, 1):
                        kt = 2 * g + j
                        o = kt * KTS - qt * QTS
                        lo = max(0, o)
                        los.append(lo)
                        nc.tensor.matmul(
                            sc[:, j, lo:QTS],
                            kt2[:, kt * KTS : (kt + 1) * KTS],
                            qh[:, qt * QTS + lo : (qt + 1) * QTS],
                            start=True,
                            stop=True,
                        )
                    # exp (clipped); diagonal windows get 0/1 lower-tri mask
                    if los[0] == 0 and los[1] == 0 and 2 * g + 1 < 4 * qt:
                        nc.scalar.activation(e[:, :, :], sc[:, :, :], Exp)
                    else:
                        for j in (0, 1):
                            nc.scalar.activation(
                                e[:, j, los[j] : QTS], sc[:, j, los[j] : QTS], Exp
                            )
                    for j in (0, 1):
                        kt = 2 * g + j
                        o = kt * KTS - qt * QTS
                        if o >= 0:
                            if mask_split == 2:
                                eng = nc.vector
                            else:
                                eng = (
                                    nc.vector
                                    if (mask_split and j == 0)
                                    else nc.gpsimd
                                )
                            eng.tensor_tensor(
                                e[:, j, o : o + KTS],
                                e[:, j, o : o + KTS],
                                tri_sb[:],
                                mult_op,
                            )
                    return los

                def emit_pv(h, g, e, los, ops):
                    for j in (0, 1):
                        kt = 2 * g + j
                        lo = los[j]
                        nc.tensor.matmul(
                            ops[:, lo:QTS],
                            V_sb[:, kt, :],
                            e[:, j, lo:QTS],
                            start=(kt == 0),
                            stop=(kt == n_kt - 1),
                        )

                for pair in (0, 1):
                    heads = (2 * pair, 2 * pair + 1)
                    ops = {}
                    pend = {}  # (h, g) -> (e, los) awaiting PV
                    for h in heads:
                        ops[h] = psops.tile(
                            [128, QTS], f32, tag="ops", name=f"ops{h}"
                        )
                    for g in range(G + lag):
                        for h in heads:
                            if g < G:
                                sc = pssc.tile([128, 2, QTS], f32, tag="sc")
                                e = epool.tile([128, 2, QTS], bf16, tag="e")
                                los = emit_scores(h, g, sc, e)
                                pend[(h, g)] = (e, los)
                        for h in heads:
                            if g >= lag:
                                e, los = pend.pop((h, g - lag))
                                emit_pv(h, g - lag, e, los, ops[h])
                        pop_filler(1)
                    # normalize: evacuate O^T+sumexp to SBUF, recip, broadcast,
                    # scale into OT (broadcast + scale on Pool, off the DVE
                    # critical path)
                    for h in heads:
                        hb = (h % 2) * 64
                        osum = rpool.tile([72, QTS], f32, tag="osum")
                        rt = rpool.tile([1, QTS], f32, tag="rt")
                        # recip reads the sumexp row straight from PSUM so it
                        # doesn't serialize behind the O^T evacuation
                        nc.vector.reciprocal(rt[:], ops[h][64:65, :])
                        if osum_act:
                            nc.scalar.activation(osum[0:64, :], ops[h][0:64, :], Copy)
                        else:
                            nc.vector.tensor_copy(osum[0:64, :], ops[h][0:64, :])
                        bsb = rpool.tile([64, QTS], f32, tag="bsb")
                        nc.gpsimd.partition_broadcast(bsb[:], rt[:])
                        eng = nc.gpsimd if norm_pool else nc.vector
                        eng.tensor_tensor(
                            OT_sb[hb : hb + 64, h // 2, qsl],
                            osum[0:64, :],
                            bsb[:],
                            mult_op,
                        )
                    pop_filler(1)

            # ---------------- schedule ----------------
            mark("x0_load")
            # ones column / zero pad for the PV lhsT of both batches, issued
            # first so nothing downstream waits on the Pool counter
            for b in (0, 1):
                V_b = get_batch_tiles(b)[("V", b)]
                nc.gpsimd.memset(V_b[:, :, 64:128], 0.0)
                nc.gpsimd.memset(V_b[:, :, 64:65], 1.0)
            # first chunk split in half so the first projection matmuls can
            # begin while the second half is still in flight; cos/sin for the
            # first token tile split off so RoPE isn't blocked behind the
            # full tables
            xT0 = get_batch_tiles(0)[("xT", 0)]
            nc.sync.dma_start(
                xT0[:, 0:8, 0:QTS],
                xT_d[0, 0:1024, 0:QTS].rearrange("(j p) c -> p j c", p=128),
            )
            nc.sync.dma_start(
                xT0[:, 8:16, 0:QTS],
                xT_d[0, 1024:2048, 0:QTS].rearrange("(j p) c -> p j c", p=128),
            )
            nc.sync.dma_start(cos_sb[:, 0:QTS], cos_d[:, 0:QTS])
            nc.sync.dma_start(sin_sb[:, 0:QTS], sin_d[:, 0:QTS])
            nc.sync.dma_start(wq_sb[:, :, :], wq_d[:, :])
            nc.sync.dma_start(tri_sb[:], tri_d[:])
            nc.sync.dma_start(cos_sb[:, QTS:s], cos_d[:, QTS:s])
            nc.sync.dma_start(sin_sb[:, QTS:s], sin_d[:, QTS:s])
            for n in range(1, n_qt):
                load_x_chunk(0, n)
            nc.sync.dma_start(wo_sb[:, :, :], wo_d[:, :])
            # projections run one token tile ahead of attention so the RoPE /
            # V-transpose chains finish during the previous attention block;
            # b1's first projections interleave into b0's last attention block
            for b in (0, 1):
                for n in range(n_qt):
                    if not (b == 1 and n == 0):
                        kv_proj_block(b, n)
                        q_proj_block(b, n, 0)
                        pop_filler(1)
                        q_proj_block(b, n, 1)
                    pop_filler(1)
                    if n > 0:
                        attn_qtile(b, n - 1, evac_parity=n % 2)
                        queue_wo(b, n - 1)
                    if b == 0:
                        # after attn so the next kv block's V transposes are
                        # not queued behind these long transfers
                        load_x_chunk(1, n, split=4)
                if b == 0:
                    pre_fillers.append(lambda: kv_proj_block(1, 0))
                    pre_fillers.append(lambda: q_proj_block(1, 0, 0))
                    pre_fillers.append(lambda: q_proj_block(1, 0, 1))
                attn_qtile(b, n_qt - 1, evac_parity=0)
                queue_wo(b, n_qt - 1)
                while pre_fillers:
                    pre_fillers.popleft()()
            mark("drain")
            # wave drain: all attention PSUM pools are idle now, so run the
            # first-contraction matmuls of several units back-to-back (they
            # only need the pair-0 OT block), then the second halves + evacs
            while wo_queue:
                wave = []
                for _ in range(6):
                    if wo_queue:
                        wave.append(wo_queue.popleft())
                parts = []
                for i, (b_, mt_, nw_) in enumerate(wave):
                    pool, tg = ((pssc, "sc"), (psops, "ops"), (psw, "w"))[i % 3]
                    ps = pool.tile([128, QTS], f32, tag=tg, name=f"dr{i}")
                    OT_sb = tiles[("OT", b_)]
                    msl = slice(mt_ * 128, (mt_ + 1) * 128)
                    nsl = slice(nw_ * QTS, (nw_ + 1) * QTS)
                    nc.tensor.matmul(
                        ps[:], OT_sb[:, 0, msl], wo_sb[:, 0, nsl],
                        start=True, stop=False,
                    )
                    parts.append((ps, b_, mt_, nw_, msl, nsl))
                for i, (ps, b_, mt_, nw_, msl, nsl) in enumerate(parts):
                    OT_sb = tiles[("OT", b_)]
                    nc.tensor.matmul(
                        ps[:], OT_sb[:, 1, msl], wo_sb[:, 1, nsl],
                        start=False, stop=True,
                    )
                    osb = tiles.get(("osb", b_, mt_))
                    if osb is None:
                        osb = opool.tile(
                            [128, d], bf16, tag="osb", name=f"osb{b_}_{mt_}"
                        )
                        tiles[("osb", b_, mt_)] = osb
                    if i % 2 == 1:
                        nc.scalar.activation(osb[:, nsl], ps[:], Copy)
                    else:
                        nc.vector.tensor_copy(osb[:, nsl], ps[:])
                    if nw_ == 1:
                        nc.sync.dma_start(
                            part_d[b_ * s + mt_ * 128 : b_ * s + (mt_ + 1) * 128, 0:1024],
                            osb[:, 0:1024],
                        )
                    if nw_ == d // QTS - 1:
                        nc.sync.dma_start(
                            part_d[b_ * s + mt_ * 128 : b_ * s + (mt_ + 1) * 128, 1024:d],
                            osb[:, 1024:d],
                        )
                        del tiles[("osb", b_, mt_)]
    mark("end")
    nc.compile()
    return nc


# ---------------- host-side sharding ----------------

_PERM = np.concatenate([np.arange(0, HD, 2), np.arange(1, HD, 2)])  # evens, odds


def make_core_inputs(x, freqs_cos, freqs_sin, wq, wk, wv, wo, s=S, d=D):
    """Build per-core input maps (list of dicts, one per core)."""
    xT = np.ascontiguousarray(np.transpose(x, (0, 2, 1))).astype(BF16)  # [B, D, S]

    cosT = np.ascontiguousarray(freqs_cos.T)  # [32, S]
    sinT = np.ascontiguousarray(freqs_sin.T)
    cosb = np.tile(np.concatenate([cosT, cosT], axis=0), (2, 1)).astype(BF16)  # [128,S]
    sinb = np.tile(np.concatenate([-sinT, sinT], axis=0), (2, 1)).astype(BF16)

    p = np.arange(128)[:, None]
    c = np.arange(128)[None, :]
    tri128 = np.where(c >= p, 1.0, 0.0).astype(BF16)

    scale = 1.0 / math.sqrt(HD)
    in_maps = []
    for cidx in range(N_CORES):
        wq_c = np.concatenate(
            [
                wq[:, (4 * cidx + h) * HD : (4 * cidx + h + 1) * HD][:, _PERM]
                for h in range(HQ)
            ],
            axis=1,
        ) * scale
        wk_c = wk[:, cidx * HD : (cidx + 1) * HD][:, _PERM]
        wv_c = wv[:, cidx * HD : (cidx + 1) * HD]
        wkv_c = np.concatenate([wk_c, wv_c], axis=1)  # [D, 128]
        wo_c = wo[4 * cidx * HD : (4 * cidx + HQ) * HD, :]  # [256, D]
        # rearrange into SBUF layout [partition, kt*cols] so each load is one
        # contiguous-per-partition DMA
        wkv_r = np.ascontiguousarray(
            wkv_c.reshape(16, 128, 128).transpose(1, 0, 2).reshape(128, -1)
        ).astype(BF16)
        wq_r = np.ascontiguousarray(
            wq_c.reshape(16, 128, 256).transpose(1, 0, 2).reshape(128, -1)
        ).astype(BF16)
        wo_r = np.ascontiguousarray(
            wo_c.reshape(2, 128, D).transpose(1, 0, 2).reshape(128, -1)
        ).astype(BF16)
        in_maps.append(
            {
                "xT": xT,
                "wkv_r": wkv_r,
                "wq_r": wq_r,
                "wo_r": wo_r,
                "cosb": cosb,
                "sinb": sinb,
                "tri128": tri128,
            }
        )
    return in_maps


_NC_CACHE = {}


def kernel(x, freqs_cos, freqs_sin, wq, wk, wv, wo):
    from concourse.bass_utils import run_bass_kernel_spmd

    x = np.asarray(x, np.float32)
    freqs_cos = np.asarray(freqs_cos, np.float32)
    freqs_sin = np.asarray(freqs_sin, np.float32)
    wq = np.asarray(wq, np.float32)
    wk = np.asarray(wk, np.float32)
    wv = np.asarray(wv, np.float32)
    wo = np.asarray(wo, np.float32)

    if "nc" not in _NC_CACHE:
        _NC_CACHE["nc"] = build_program()
    nc = _NC_CACHE["nc"]

    in_maps = make_core_inputs(x, freqs_cos, freqs_sin, wq, wk, wv, wo)
    res = run_bass_kernel_spmd(nc, in_maps, list(range(N_CORES)))
    acc = np.zeros((B * S, D), np.float32)
    for r in res.results:
        acc += np.asarray(r["part"], np.float32)
    return acc.reshape(B, S, D).astype(BF16)
